# revision 28
# baseline (speedup 1.0000x reference)
"""Multi-head causal self-attention (B=2, T=2048, D=1024, H=16, Dh=64) on 8 TRN2 cores.

Sharding: data-parallel over batch (2 groups of 4 cores), tensor-parallel over
heads within a group (4 heads/core). Each core computes its 4 heads'
QKV projection + causal flash attention + its slice of the output projection;
the host sums the 4 partial outputs per batch (bf16 partials, f32 sum).

Design (per core):
  - All matmuls bf16 (1 cyc/row at every N; fp32r pays 4x below N=256),
    except: (a) the score matmul - q/k are stored fp8e4m3 in a [32,2]
    DoubleRow packing, so S^T runs at 0.5 cyc/row (2x PE); (b) the
    q-projection itself - x and Wq ship as fp8 and contract DoubleRow
    (4 chunks of K=256), another 2x. Wq is pre-scaled by 32 on the host to
    lift it out of e4m3's subnormal range (compensated in the exp scale;
    the mask bias scales to -6144). The wq/wk columns are pre-permuted on
    the host so the projection psum lands directly in the packed S layout
    (the psum->sbuf copy is partition-preserving). k/v/out projections and
    PV stay bf16 - fp8 there fails the 2e-2 gate. Measured rel-err ~1.4e-2.
  - Attention runs transposed (S^T[tk, tq] blocks, 512-wide tq slices);
    softmax sums come free via a ones-column appended to V, so PV emits
    [y^T; sums] per head with no extra matmul cost (cost = moving dim only).
  - Causal mask is an additive bias matmul (identity lhsT x [-192 staircase]
    bf16) into the score psum on the mixed diagonal 128-block only; exp then
    yields e^-24 ~ 0 there. No vector-engine work on the exp->PV critical
    path. S matmuls/exps are column-restricted to the causal region;
    diagonal se tiles are dedicated per (pair, r) with their fully-masked
    left columns zeroed once at startup.
  - Head-pair S blocks land in one 2-bank psum tile => a single fused exp
    per (pair, ii) halves the ACT instruction count. ACT runs exp only;
    q/k/v psum copies are on DVE; softmax-sum broadcasts on gpsimd.
  - Emission is a deadline-driven schedule: every projection/out-projection
    matmul group becomes a filler with (deadline, ready) over a global unit
    order, flushed just-in-time or paced evenly, so the in-order PE queue
    never parks behind an exp-blocked score matmul. Out-projection drifts to
    the late (filler-starved) slices; the final slice normalizes in column
    chunks and alternates its psum copies between ACT and DVE to pipeline
    the tail stores.
  - Preload uses few large DMAs in consumption order (the DMA bus
    round-robins across queues, so small DMAs would let low-priority
    transfers steal bus turns from the critical first projection).
"""
import sys

import numpy as np

for _p in ("/opt/trn_rl_repo", "/root/.axon_site/_ro/trn_rl_repo"):
    if _p not in sys.path:
        try:
            import concourse  # noqa: F401
            break
        except ImportError:
            sys.path.append(_p)

import ml_dtypes  # noqa: E402
import concourse.bass as bass  # noqa: E402
import concourse.tile as tile  # noqa: E402
from concourse import bacc, mybir  # noqa: E402
from concourse.bass_utils import run_bass_kernel_spmd  # noqa: E402

P = 128
T = 2048
D = 1024
NH = 4          # heads per core
DH = 64
F = NH * DH     # per-core head features (256)
DC = D // P     # 8 contraction chunks
TJ = T // 512   # 4 tq slices
TC = T // P     # 16 tk chunks
N_CORES = 8
F32 = mybir.dt.float32
BF = mybir.dt.bfloat16
F8 = mybir.dt.float8e4
AF = mybir.ActivationFunctionType
DRM = mybir.MatmulPerfMode.DoubleRow
BFNP = ml_dtypes.bfloat16
F8NP = ml_dtypes.float8_e4m3

QK_FP8 = True  # q/k in fp8e4m3 + DoubleRow score matmul (2x PE on scores)

QK_DT = F8 if QK_FP8 else BF
WQ_SCALE = 32.0  # lifts Wq out of e4m3 subnormals; folded into exp scale
MASK_BIAS = -192.0 * WQ_SCALE  # exp(scale/WQ_SCALE * bias) = e^-24 ~ 4e-11


def build():
    nc = bacc.Bacc("TRN2", target_bir_lowering=False, debug=False, num_devices=N_CORES)
    xT = nc.dram_tensor("xT", [D, T], BF, kind="ExternalInput").ap()
    xT8 = nc.dram_tensor("xT8", [D, T], F8, kind="ExternalInput").ap()
    wqT = nc.dram_tensor("wqT", [D, F], F8, kind="ExternalInput").ap()
    wkT = nc.dram_tensor("wkT", [D, F], BF, kind="ExternalInput").ap()
    wvT = nc.dram_tensor("wvT", [D, F], BF, kind="ExternalInput").ap()
    woT = nc.dram_tensor("woT", [F, D], BF, kind="ExternalInput").ap()
    ident = nc.dram_tensor("ident", [P, P], BF, kind="ExternalInput").ap()
    maskb = nc.dram_tensor("maskb", [P, P], BF, kind="ExternalInput").ap()
    out = nc.dram_tensor("out", [T, D], BF, kind="ExternalOutput").ap()

    scale = 1.0 / np.sqrt(DH) / WQ_SCALE

    with tile.TileContext(nc) as tc:
        with (
            tc.tile_pool(name="weights", bufs=1) as wpool,
            tc.tile_pool(name="persist", bufs=1) as persist,
            tc.tile_pool(name="x", bufs=2) as xpool,
            tc.tile_pool(name="sexp", bufs=6) as sepool,
            tc.tile_pool(name="small", bufs=10) as small,
            tc.tile_pool(name="outsb", bufs=4) as opool,
            tc.tile_pool(name="ps_s", bufs=2, space="PSUM") as ps_s,
            tc.tile_pool(name="ps_y", bufs=2, space="PSUM") as ps_y,
            tc.tile_pool(name="ps_ao", bufs=2, space="PSUM") as ps_ao,
        ):
            wq_sb = wpool.tile([P, DC // 2, 2, F], F8)
            wk_sb = wpool.tile([P, DC, F], BF)
            wv_sb = wpool.tile([P, DC, F], BF)
            wo_sb = wpool.tile([P, 2, D], BF)
            id_sb = wpool.tile([P, P], BF)
            mb_sb = wpool.tile([P, P], BF)
            wq_r = wqT.rearrange("(o i p) f -> p o i f", p=P, i=2)
            wk_r = wkT.rearrange("(o p) f -> p o f", p=P)
            wv_r = wvT.rearrange("(o p) f -> p o f", p=P)
            wo_r = woT.rearrange("(g p) e -> p g e", p=P)
            xT_r = xT.rearrange("(o p) t -> p o t", p=P)
            xT8_r = xT8.rearrange("(o i p) t -> p o i t", p=P, i=2)

            qT_sb = persist.tile([P, 2, T], QK_DT)
            kT_sb = persist.tile([P, 2, T], QK_DT)
            v_sb = persist.tile([P, NH, TC, DH + 1], BF)
            yT_sb = persist.tile([P, 2, T], BF)
            # dedicated diagonal se tiles per (pair c, r>=1): left cols hold
            # persistent zeros written once below.
            sediag = {
                (c, r): persist.tile([P, 2, 512], BF, name=f"sed_{c}_{r}")
                for c in range(2)
                for r in range(1, 4)
            }

            x_tiles = {}
            x8_tiles = {}

            def load_x(j):
                x_sb = xpool.tile([P, DC, 512], BF, tag="x", name=f"x_{j}")
                nc.sync.dma_start(x_sb[:], xT_r[:, :, 512 * j : 512 * (j + 1)])
                x_tiles[j] = x_sb
                x8_sb = xpool.tile([P, DC // 2, 2, 512], F8, tag="x8", name=f"x8_{j}")
                nc.sync.dma_start(x8_sb[:], xT8_r[:, :, :, 512 * j : 512 * (j + 1)])
                x8_tiles[j] = x8_sb

            # Preload in consumption order, few large DMAs: the DMA bus
            # round-robins across queues, so many small DMAs let low-priority
            # transfers steal bus turns from the critical first projection.
            x0_sb = xpool.tile([P, DC, 512], BF, tag="x", name="x_0")
            x_tiles[0] = x0_sb
            x80_sb = xpool.tile([P, DC // 2, 2, 512], F8, tag="x8", name="x8_0")
            x8_tiles[0] = x80_sb
            nc.sync.dma_start(wq_sb[:], wq_r[:])
            nc.sync.dma_start(x80_sb[:], xT8_r[:, :, :, 0:512])
            nc.sync.dma_start(wk_sb[:, 0:4], wk_r[:, 0:4])
            nc.sync.dma_start(x0_sb[:, 0:4], xT_r[:, 0:4, 0:512])
            nc.sync.dma_start(wk_sb[:, 4:8], wk_r[:, 4:8])
            nc.sync.dma_start(x0_sb[:, 4:8], xT_r[:, 4:8, 0:512])
            nc.sync.dma_start(wv_sb[:], wv_r[:])
            nc.sync.dma_start(id_sb[:], ident)
            nc.sync.dma_start(mb_sb[:], maskb)
            load_x(1)
            nc.sync.dma_start(wo_sb[:], wo_r[:])

            # V's softmax-sum ones column + persistent zeros in the
            # fully-masked left region of diagonal se tiles.
            nc.gpsimd.memset(v_sb[:, :, :, DH : DH + 1], 1.0)
            for (c, r), t_ in sediag.items():
                nc.gpsimd.memset(t_[:, :, 0 : 128 * r], 0.0)

            def proj_groups(j):
                jsl = slice(512 * j, 512 * (j + 1))
                x_sb = x_tiles[j]
                groups = []
                x8_sb = x8_tiles[j]
                for c in range(2):
                    def g(c=c):
                        pt = ps_ao.tile([P, 512], F32, tag="ao")
                        for o in range(DC // 2):
                            nc.tensor.matmul(
                                pt[:],
                                wq_sb[:, o, :, 128 * c : 128 * (c + 1)],
                                x8_sb[:, o, :, :],
                                start=(o == 0),
                                stop=(o == DC // 2 - 1),
                                perf_mode=DRM,
                            )
                        nc.vector.tensor_copy(qT_sb[:, c, jsl], pt[:])
                    groups.append(g)
                for c in range(2):
                    def g(c=c):
                        pt = ps_ao.tile([P, 512], F32, tag="ao")
                        for o in range(DC):
                            nc.tensor.matmul(
                                pt[:],
                                wk_sb[:, o, 128 * c : 128 * (c + 1)],
                                x_sb[:, o, :],
                                start=(o == 0),
                                stop=(o == DC - 1),
                            )
                        nc.vector.tensor_copy(kT_sb[:, c, jsl], pt[:])
                    groups.append(g)
                for i in range(4):
                    def g(i=i):
                        pt = ps_ao.tile([P, 512], F32, tag="ao")
                        for o in range(DC):
                            nc.tensor.matmul(
                                pt[:, :F],
                                x_sb[:, o, 128 * i : 128 * (i + 1)],
                                wv_sb[:, o, :],
                                start=(o == 0),
                                stop=(o == DC - 1),
                            )
                        nc.vector.tensor_copy(
                            v_sb[:, :, 4 * j + i, 0:DH],
                            pt[:, :F].rearrange("p (h d) -> p h d", h=NH),
                        )
                    groups.append(g)
                return groups

            def outproj_groups(j, copy_on_act=False):
                groups = []
                for tb in range(4 * j, 4 * (j + 1)):
                    for eb in range(2):
                        def g(tb=tb, eb=eb):
                            pt = ps_ao.tile([P, 512], F32, tag="ao")
                            for g2 in range(2):
                                nc.tensor.matmul(
                                    pt[:],
                                    yT_sb[:, g2, 128 * tb : 128 * (tb + 1)],
                                    wo_sb[:, g2, 512 * eb : 512 * (eb + 1)],
                                    start=(g2 == 0),
                                    stop=(g2 == 1),
                                )
                            osb = opool.tile([P, 512], BF, tag="osb")
                            if copy_on_act and (tb + eb) % 2 == 0:
                                nc.scalar.copy(osb[:], pt[:])
                            else:
                                nc.vector.tensor_copy(osb[:], pt[:])
                            nc.sync.dma_start(
                                out[128 * tb : 128 * (tb + 1), 512 * eb : 512 * (eb + 1)],
                                osb[:],
                            )
                        groups.append(g)
                return groups

            def attn_unit(j, c, ii, nii, ypair, mid):
                r = ii - 4 * j
                col0 = 128 * r if r > 0 else 0
                qsl = slice(512 * j + col0, 512 * (j + 1))
                diag = r >= 0
                sps = ps_s.tile([P, 2, 512], F32, tag="s")
                for t in range(2):
                    if QK_FP8:
                        h = 2 * c + t
                        nc.tensor.matmul(
                            sps[:, t, col0:],
                            kT_sb[32 * h : 32 * h + 32, :, 128 * ii : 128 * (ii + 1)],
                            qT_sb[32 * h : 32 * h + 32, :, qsl],
                            start=True,
                            stop=not diag,
                            perf_mode=DRM,
                            tile_position=(32 * h, 0),
                        )
                    else:
                        hp = 64 * t
                        nc.tensor.matmul(
                            sps[:, t, col0:],
                            kT_sb[hp : hp + DH, c, 128 * ii : 128 * (ii + 1)],
                            qT_sb[hp : hp + DH, c, qsl],
                            start=True,
                            stop=not diag,
                        )
                    if diag:
                        # additive causal mask on the mixed 128-block
                        nc.tensor.matmul(
                            sps[:, t, col0 : col0 + 128],
                            id_sb[:],
                            mb_sb[:],
                            start=False,
                            stop=True,
                        )
                se = (
                    sediag[(c, r)]
                    if r >= 1
                    else sepool.tile([P, 2, 512], BF, tag="se")
                )
                nc.scalar.activation(
                    se[:, :, col0:], sps[:, :, col0:], AF.Exp, scale=scale
                )
                for f in mid:
                    f()
                for t in range(2):
                    nc.tensor.matmul(
                        ypair[t][:],
                        v_sb[:, 2 * c + t, ii, :],
                        se[:, t, :],
                        start=(ii == 0),
                        stop=(ii == nii - 1),
                    )

            def normalize(j, c, ypair):
                # All recips first (DVE), broadcasts next (Pool, overlaps the
                # recips), muls last — so the in-order DVE queue never parks
                # behind a Pool broadcast. Chunked on the last slice so
                # outproj(3) tb-group tb starts as soon as its yT chunk lands.
                nchunk = 2 if j == TJ - 1 else 1
                w = 512 // nchunk
                bsbs = {}
                for q4 in range(nchunk):
                    for t in range(2):
                        qs = slice(w * q4, w * (q4 + 1))
                        rec = small.tile([1, w], F32, tag=f"rec{nchunk}")
                        nc.vector.reciprocal(rec[:], ypair[t][DH : DH + 1, qs])
                        bsb = small.tile([DH, w], F32, tag=f"bsb{nchunk}")
                        nc.gpsimd.partition_broadcast(bsb[:], rec[:])
                        bsbs[(t, q4)] = bsb
                for q4 in range(nchunk):
                    for t in range(2):
                        qs = slice(w * q4, w * (q4 + 1))
                        ys = slice(512 * j + w * q4, 512 * j + w * (q4 + 1))
                        nc.vector.tensor_mul(
                            yT_sb[64 * t : 64 * t + DH, c, ys],
                            ypair[t][0:DH, qs],
                            bsbs[(t, q4)],
                        )

            # Remaining x slices: queued behind the preload on the same DMA
            # ring, arriving well before their projections need them.
            load_x(2)
            load_x(3)

            # ---- deadline-driven emission schedule ----
            # One global unit order; every proj/outproj matmul group becomes
            # a filler with (deadline unit, ready unit). Deadline fillers are
            # flushed right before their unit; the rest pace out evenly so
            # the in-order PE queue always has independent work between
            # exp-blocked attention steps. proj(0) interleaves into the j=0
            # units, so the ACT exp pipeline starts ~14us earlier.
            sched = []
            for j in range(TJ):
                for c in range(2):
                    for ii in range(4 * j + 4):
                        sched.append((j, c, ii))
            idx = {u: s for s, u in enumerate(sched)}
            NU = len(sched)

            fillers = []  # (deadline, ready, fn)
            for j in range(TJ):
                pg = proj_groups(j)  # [q-c0, q-c1, k-c0, k-c1, v0..v3] halves
                for c in range(2):
                    # fp8 layout: dim1 of qT/kT is the dh-tile, so every S
                    # matmul reads BOTH projection chunks => both must land
                    # before the slice's first unit.
                    d = idx[(j, 0, 0)] if QK_FP8 else idx[(j, c, 0)]
                    fillers.append((d, 0, pg[c]))
                    fillers.append((d, 0, pg[2 + c]))
                for i4 in range(4):
                    fillers.append((idx[(j, 0, 4 * j + i4)], 0, pg[4 + i4]))
            for j in range(TJ - 1):
                ready = idx[(j, 1, 4 * j + 3)] + 1
                for g in outproj_groups(j):
                    fillers.append((NU, ready, g))
            fillers.sort(key=lambda f: (f[0], f[1]))
            NF = len(fillers)

            fi = 0
            ypairs = {}
            for s, (j, c, ii) in enumerate(sched):
                nii = 4 * j + 4
                while fi < NF and fillers[fi][0] <= s:
                    fillers[fi][2]()
                    fi += 1
                if ii == 0:
                    ypairs[c] = [
                        ps_y.tile([DH + 1, 512], F32, tag="y", name=f"y_{j}_{c}_{t}")
                        for t in range(2)
                    ]
                attn_unit(j, c, ii, nii, ypairs[c], [])
                if ii == nii - 1:
                    normalize(j, c, ypairs[c])
                target = (s + 1) * NF // NU
                while fi < min(target, NF) and fillers[fi][1] <= s:
                    fillers[fi][2]()
                    fi += 1
            while fi < NF:
                fillers[fi][2]()
                fi += 1
            for g in outproj_groups(TJ - 1, copy_on_act=True):
                g()
    nc.compile()
    return nc


def make_ident() -> np.ndarray:
    return np.eye(P, dtype=np.float32)


def make_maskb() -> np.ndarray:
    q = np.arange(P)[None, :]
    p = np.arange(P)[:, None]
    return np.ascontiguousarray(np.where(q >= p, 0.0, MASK_BIAS).astype(np.float32))


def qk_col_perm() -> np.ndarray:
    """Column permutation of wq/wk so the projection psum partitions land in
    the fp8 DoubleRow [32, 2] packing: matmul chunk c, out partition
    p = 32*h + p'  <->  feature f = 64*h + 32*c + p'."""
    perm = []
    for c in range(2):
        for jj in range(128):
            perm.append(64 * (jj // 32) + 32 * c + (jj % 32))
    return np.asarray(perm)


def shard_inputs(x, Wqkv, Wout):
    ident = make_ident().astype(BFNP)
    maskb = make_maskb().astype(BFNP)
    perm = qk_col_perm() if QK_FP8 else np.arange(F)
    in_maps = []
    for core in range(N_CORES):
        b, g = core // 4, core % 4
        sl = slice(F * g, F * (g + 1))
        wq = np.ascontiguousarray(Wqkv[sl, :].T[:, perm])
        wk = np.ascontiguousarray(Wqkv[D:][sl, :].T[:, perm])
        in_maps.append(
            {
                "xT": np.ascontiguousarray(x[b].T).astype(BFNP),
                "xT8": np.ascontiguousarray(x[b].T).astype(F8NP),
                "wqT": (wq * WQ_SCALE).astype(F8NP),
                "wkT": wk.astype(BFNP),
                "wvT": np.ascontiguousarray(Wqkv[2 * D:][sl, :].T).astype(BFNP),
                "woT": np.ascontiguousarray(Wout[:, sl].T).astype(BFNP),
                "ident": ident,
                "maskb": maskb,
            }
        )
    return in_maps


_NC_CACHE = None


def kernel(x, Wqkv, Wout):
    global _NC_CACHE
    x = np.asarray(x, dtype=np.float32)
    Wqkv = np.asarray(Wqkv, dtype=np.float32)
    Wout = np.asarray(Wout, dtype=np.float32)
    if _NC_CACHE is None:
        _NC_CACHE = build()
    nc = _NC_CACHE
    in_maps = shard_inputs(x, Wqkv, Wout)
    res = run_bass_kernel_spmd(nc, in_maps, core_ids=list(range(N_CORES)))
    outs = [res.results[c]["out"].astype(np.float32) for c in range(N_CORES)]
    return np.stack(
        [outs[0] + outs[1] + outs[2] + outs[3], outs[4] + outs[5] + outs[6] + outs[7]]
    )


# revision 30
# speedup vs baseline: 1.0139x; 1.0139x over previous
"""Multi-head causal self-attention (B=2, T=2048, D=1024, H=16, Dh=64) on 8 TRN2 cores.

Sharding: data-parallel over batch (2 groups of 4 cores), tensor-parallel over
heads within a group (4 heads/core). Each core computes its 4 heads'
QKV projection + causal flash attention + its slice of the output projection;
the host sums the 4 partial outputs per batch (bf16 partials, f32 sum).

Design (per core):
  - All matmuls bf16 (1 cyc/row at every N; fp32r pays 4x below N=256),
    except: (a) the score matmul - q/k are stored fp8e4m3 in a [32,2]
    DoubleRow packing, so S^T runs at 0.5 cyc/row (2x PE); (b) the
    q-projection itself - x and Wq ship as fp8 and contract DoubleRow
    (4 chunks of K=256), another 2x. Wq is pre-scaled by 32 on the host to
    lift it out of e4m3's subnormal range (compensated in the exp scale;
    the mask bias scales to -6144). The wq/wk columns are pre-permuted on
    the host so the projection psum lands directly in the packed S layout
    (the psum->sbuf copy is partition-preserving). k/v/out projections and
    PV stay bf16 - fp8 there fails the 2e-2 gate. Measured rel-err ~1.4e-2.
  - Attention runs transposed (S^T[tk, tq] blocks, 512-wide tq slices);
    softmax sums come free via a ones-column appended to V, so PV emits
    [y^T; sums] per head with no extra matmul cost (cost = moving dim only).
  - Causal mask is an additive bias matmul (identity lhsT x [-192 staircase]
    bf16) into the score psum on the mixed diagonal 128-block only; exp then
    yields e^-24 ~ 0 there. No vector-engine work on the exp->PV critical
    path. S matmuls/exps are column-restricted to the causal region;
    diagonal se tiles are dedicated per (pair, r) with their fully-masked
    left columns zeroed once at startup.
  - Head-pair S blocks land in one 2-bank psum tile => a single fused exp
    per (pair, ii) halves the ACT instruction count. ACT runs exp only;
    q/k/v psum copies are on DVE; softmax-sum broadcasts on gpsimd.
  - Emission is a deadline-driven schedule: every projection/out-projection
    matmul group becomes a filler with (deadline, ready) over a global unit
    order, flushed just-in-time or paced evenly, so the in-order PE queue
    never parks behind an exp-blocked score matmul. Out-projection drifts to
    the late (filler-starved) slices; the final slice normalizes in column
    chunks and alternates its psum copies between ACT and DVE to pipeline
    the tail stores.
  - Preload uses few large DMAs in consumption order (the DMA bus
    round-robins across queues, so small DMAs would let low-priority
    transfers steal bus turns from the critical first projection).
"""
import sys

import numpy as np

for _p in ("/opt/trn_rl_repo", "/root/.axon_site/_ro/trn_rl_repo"):
    if _p not in sys.path:
        try:
            import concourse  # noqa: F401
            break
        except ImportError:
            sys.path.append(_p)

import ml_dtypes  # noqa: E402
import concourse.bass as bass  # noqa: E402
import concourse.tile as tile  # noqa: E402
from concourse import bacc, mybir  # noqa: E402
from concourse.bass_utils import run_bass_kernel_spmd  # noqa: E402

P = 128
T = 2048
D = 1024
NH = 4          # heads per core
DH = 64
F = NH * DH     # per-core head features (256)
DC = D // P     # 8 contraction chunks
TJ = T // 512   # 4 tq slices
TC = T // P     # 16 tk chunks
N_CORES = 8
F32 = mybir.dt.float32
BF = mybir.dt.bfloat16
F8 = mybir.dt.float8e4
AF = mybir.ActivationFunctionType
DRM = mybir.MatmulPerfMode.DoubleRow
BFNP = ml_dtypes.bfloat16
F8NP = ml_dtypes.float8_e4m3

QK_FP8 = True  # q/k in fp8e4m3 + DoubleRow score matmul (2x PE on scores)

QK_DT = F8 if QK_FP8 else BF
WQ_SCALE = 32.0  # lifts Wq out of e4m3 subnormals; folded into exp scale
MASK_BIAS = -192.0 * WQ_SCALE  # exp(scale/WQ_SCALE * bias) = e^-24 ~ 4e-11


def build():
    nc = bacc.Bacc("TRN2", target_bir_lowering=False, debug=False, num_devices=N_CORES)
    xT = nc.dram_tensor("xT", [D, T], BF, kind="ExternalInput").ap()
    xT8 = nc.dram_tensor("xT8", [D, T], F8, kind="ExternalInput").ap()
    wqT = nc.dram_tensor("wqT", [D, F], F8, kind="ExternalInput").ap()
    wkT = nc.dram_tensor("wkT", [D, F], BF, kind="ExternalInput").ap()
    wvT = nc.dram_tensor("wvT", [D, F], BF, kind="ExternalInput").ap()
    woT = nc.dram_tensor("woT", [F, D], BF, kind="ExternalInput").ap()
    ident = nc.dram_tensor("ident", [P, P], BF, kind="ExternalInput").ap()
    maskb = nc.dram_tensor("maskb", [P, P], BF, kind="ExternalInput").ap()
    out = nc.dram_tensor("out", [T, D], BF, kind="ExternalOutput").ap()

    scale = 1.0 / np.sqrt(DH) / WQ_SCALE

    with tile.TileContext(nc) as tc:
        with (
            tc.tile_pool(name="weights", bufs=1) as wpool,
            tc.tile_pool(name="persist", bufs=1) as persist,
            tc.tile_pool(name="x", bufs=2) as xpool,
            tc.tile_pool(name="sexp", bufs=6) as sepool,
            tc.tile_pool(name="small", bufs=10) as small,
            tc.tile_pool(name="outsb", bufs=4) as opool,
            tc.tile_pool(name="ps_s", bufs=2, space="PSUM") as ps_s,
            tc.tile_pool(name="ps_y", bufs=2, space="PSUM") as ps_y,
            tc.tile_pool(name="ps_ao", bufs=2, space="PSUM") as ps_ao,
        ):
            wq_sb = wpool.tile([P, DC // 2, 2, F], F8)
            wk_sb = wpool.tile([P, DC, F], BF)
            wv_sb = wpool.tile([P, DC, F], BF)
            wo_sb = wpool.tile([P, 2, D], BF)
            mb_sb = wpool.tile([P, 1, P], BF)
            wq_r = wqT.rearrange("(o i p) f -> p o i f", p=P, i=2)
            wk_r = wkT.rearrange("(o p) f -> p o f", p=P)
            wv_r = wvT.rearrange("(o p) f -> p o f", p=P)
            wo_r = woT.rearrange("(g p) e -> p g e", p=P)
            xT_r = xT.rearrange("(o p) t -> p o t", p=P)
            xT8_r = xT8.rearrange("(o i p) t -> p o i t", p=P, i=2)

            qT_sb = persist.tile([P, 2, T], QK_DT)
            kT_sb = persist.tile([P, 2, T], QK_DT)
            v_sb = persist.tile([P, NH, TC, DH + 1], BF)
            yT_sb = persist.tile([P, 2, T], BF)
            # dedicated diagonal se tiles per (pair c, r>=1): left cols hold
            # persistent zeros written once below.
            sediag = {
                (c, r): persist.tile([P, 2, 512], BF, name=f"sed_{c}_{r}")
                for c in range(2)
                for r in range(1, 4)
            }

            x_tiles = {}
            x8_tiles = {}

            def load_x(j):
                x_sb = xpool.tile([P, DC, 512], BF, tag="x", name=f"x_{j}")
                nc.sync.dma_start(x_sb[:], xT_r[:, :, 512 * j : 512 * (j + 1)])
                x_tiles[j] = x_sb
                x8_sb = xpool.tile([P, DC // 2, 2, 512], F8, tag="x8", name=f"x8_{j}")
                nc.sync.dma_start(x8_sb[:], xT8_r[:, :, :, 512 * j : 512 * (j + 1)])
                x8_tiles[j] = x8_sb

            # Preload in consumption order, few large DMAs: the DMA bus
            # round-robins across queues, so many small DMAs let low-priority
            # transfers steal bus turns from the critical first projection.
            x0_sb = xpool.tile([P, DC, 512], BF, tag="x", name="x_0")
            x_tiles[0] = x0_sb
            x80_sb = xpool.tile([P, DC // 2, 2, 512], F8, tag="x8", name="x8_0")
            x8_tiles[0] = x80_sb
            nc.sync.dma_start(wq_sb[:], wq_r[:])
            nc.sync.dma_start(x80_sb[:, 0:2], xT8_r[:, 0:2, :, 0:512])
            nc.sync.dma_start(x80_sb[:, 2:4], xT8_r[:, 2:4, :, 0:512])
            nc.sync.dma_start(wk_sb[:, 0:4], wk_r[:, 0:4])
            nc.sync.dma_start(x0_sb[:, 0:4], xT_r[:, 0:4, 0:512])
            nc.sync.dma_start(wk_sb[:, 4:8], wk_r[:, 4:8])
            nc.sync.dma_start(x0_sb[:, 4:8], xT_r[:, 4:8, 0:512])
            nc.sync.dma_start(wv_sb[:], wv_r[:])
            nc.sync.dma_start(mb_sb[:, 0], maskb)
            load_x(1)
            nc.sync.dma_start(wo_sb[:], wo_r[:])

            # V's softmax-sum ones column + persistent zeros in the
            # fully-masked left region of diagonal se tiles.
            nc.gpsimd.memset(v_sb[:, :, :, DH : DH + 1], 1.0)
            for (c, r), t_ in sediag.items():
                nc.gpsimd.memset(t_[:, :, 0 : 128 * r], 0.0)

            def proj_groups(j):
                jsl = slice(512 * j, 512 * (j + 1))
                x_sb = x_tiles[j]
                groups = []
                x8_sb = x8_tiles[j]
                for c in range(2):
                    def g(c=c):
                        pt = ps_ao.tile([P, 512], F32, tag="ao")
                        for o in range(DC // 2):
                            nc.tensor.matmul(
                                pt[:],
                                wq_sb[:, o, :, 128 * c : 128 * (c + 1)],
                                x8_sb[:, o, :, :],
                                start=(o == 0),
                                stop=(o == DC // 2 - 1),
                                perf_mode=DRM,
                            )
                        nc.vector.tensor_copy(qT_sb[:, c, jsl], pt[:])
                    groups.append(g)
                for c in range(2):
                    def g(c=c):
                        pt = ps_ao.tile([P, 512], F32, tag="ao")
                        for o in range(DC):
                            nc.tensor.matmul(
                                pt[:],
                                wk_sb[:, o, 128 * c : 128 * (c + 1)],
                                x_sb[:, o, :],
                                start=(o == 0),
                                stop=(o == DC - 1),
                            )
                        nc.vector.tensor_copy(kT_sb[:, c, jsl], pt[:])
                    groups.append(g)
                for i in range(4):
                    def g(i=i):
                        pt = ps_ao.tile([P, 512], F32, tag="ao")
                        for o in range(DC):
                            nc.tensor.matmul(
                                pt[:, :F],
                                x_sb[:, o, 128 * i : 128 * (i + 1)],
                                wv_sb[:, o, :],
                                start=(o == 0),
                                stop=(o == DC - 1),
                            )
                        nc.vector.tensor_copy(
                            v_sb[:, :, 4 * j + i, 0:DH],
                            pt[:, :F].rearrange("p (h d) -> p h d", h=NH),
                        )
                    groups.append(g)
                return groups

            def outproj_groups(j, copy_on_act=False):
                groups = []
                for tb in range(4 * j, 4 * (j + 1)):
                    for eb in range(2):
                        def g(tb=tb, eb=eb):
                            pt = ps_ao.tile([P, 512], F32, tag="ao")
                            for g2 in range(2):
                                nc.tensor.matmul(
                                    pt[:],
                                    yT_sb[:, g2, 128 * tb : 128 * (tb + 1)],
                                    wo_sb[:, g2, 512 * eb : 512 * (eb + 1)],
                                    start=(g2 == 0),
                                    stop=(g2 == 1),
                                )
                            osb = opool.tile([P, 512], BF, tag="osb")
                            if copy_on_act and (tb + eb) % 2 == 0:
                                nc.scalar.copy(osb[:], pt[:])
                            else:
                                nc.vector.tensor_copy(osb[:], pt[:])
                            nc.sync.dma_start(
                                out[128 * tb : 128 * (tb + 1), 512 * eb : 512 * (eb + 1)],
                                osb[:],
                            )
                        groups.append(g)
                return groups

            def attn_unit(j, c, ii, nii, ypair, mid):
                r = ii - 4 * j
                col0 = 128 * r if r > 0 else 0
                qsl = slice(512 * j + col0, 512 * (j + 1))
                diag = r >= 0
                sps = ps_s.tile([P, 2, 512], F32, tag="s")
                for t in range(2):
                    if QK_FP8:
                        h = 2 * c + t
                        nc.tensor.matmul(
                            sps[:, t, col0:],
                            kT_sb[32 * h : 32 * h + 32, :, 128 * ii : 128 * (ii + 1)],
                            qT_sb[32 * h : 32 * h + 32, :, qsl],
                            start=True,
                            stop=True,
                            perf_mode=DRM,
                            tile_position=(32 * h, 0),
                        )
                    else:
                        hp = 64 * t
                        nc.tensor.matmul(
                            sps[:, t, col0:],
                            kT_sb[hp : hp + DH, c, 128 * ii : 128 * (ii + 1)],
                            qT_sb[hp : hp + DH, c, qsl],
                            start=True,
                            stop=True,
                        )

                se = (
                    sediag[(c, r)]
                    if r >= 1
                    else sepool.tile([P, 2, 512], BF, tag="se")
                )
                nc.scalar.activation(
                    se[:, :, col0:], sps[:, :, col0:], AF.Exp, scale=scale
                )
                if diag:
                    # zero the strictly-upper part of the mixed 128-block
                    nc.vector.tensor_mul(
                        se[:, :, col0 : col0 + 128],
                        se[:, :, col0 : col0 + 128],
                        mb_sb[:, 0:1, :].to_broadcast([P, 2, P]),
                    )
                for f in mid:
                    f()
                for t in range(2):
                    nc.tensor.matmul(
                        ypair[t][:],
                        v_sb[:, 2 * c + t, ii, :],
                        se[:, t, :],
                        start=(ii == 0),
                        stop=(ii == nii - 1),
                    )

            def normalize(j, c, ypair):
                # All recips first (DVE), broadcasts next (Pool, overlaps the
                # recips), muls last — so the in-order DVE queue never parks
                # behind a Pool broadcast. Chunked on the last slice so
                # outproj(3) tb-group tb starts as soon as its yT chunk lands.
                nchunk = 2 if j == TJ - 1 else 1
                w = 512 // nchunk
                bsbs = {}
                for q4 in range(nchunk):
                    for t in range(2):
                        qs = slice(w * q4, w * (q4 + 1))
                        rec = small.tile([1, w], F32, tag=f"rec{nchunk}")
                        nc.vector.reciprocal(rec[:], ypair[t][DH : DH + 1, qs])
                        bsb = small.tile([DH, w], F32, tag=f"bsb{nchunk}")
                        nc.gpsimd.partition_broadcast(bsb[:], rec[:])
                        bsbs[(t, q4)] = bsb
                for q4 in range(nchunk):
                    for t in range(2):
                        qs = slice(w * q4, w * (q4 + 1))
                        ys = slice(512 * j + w * q4, 512 * j + w * (q4 + 1))
                        nc.vector.tensor_mul(
                            yT_sb[64 * t : 64 * t + DH, c, ys],
                            ypair[t][0:DH, qs],
                            bsbs[(t, q4)],
                        )

            # Remaining x slices: queued behind the preload on the same DMA
            # ring, arriving well before their projections need them.
            load_x(2)
            load_x(3)

            # ---- deadline-driven emission schedule ----
            # One global unit order; every proj/outproj matmul group becomes
            # a filler with (deadline unit, ready unit). Deadline fillers are
            # flushed right before their unit; the rest pace out evenly so
            # the in-order PE queue always has independent work between
            # exp-blocked attention steps. proj(0) interleaves into the j=0
            # units, so the ACT exp pipeline starts ~14us earlier.
            sched = []
            for j in range(TJ):
                for c in range(2):
                    for ii in range(4 * j + 4):
                        sched.append((j, c, ii))
            idx = {u: s for s, u in enumerate(sched)}
            NU = len(sched)

            fillers = []  # (deadline, ready, fn)
            for j in range(TJ):
                pg = proj_groups(j)  # [q-c0, q-c1, k-c0, k-c1, v0..v3] halves
                for c in range(2):
                    # fp8 layout: dim1 of qT/kT is the dh-tile, so every S
                    # matmul reads BOTH projection chunks => both must land
                    # before the slice's first unit.
                    d = idx[(j, 0, 0)] if QK_FP8 else idx[(j, c, 0)]
                    fillers.append((d, 0, pg[c]))
                    fillers.append((d, 0, pg[2 + c]))
                for i4 in range(4):
                    fillers.append((idx[(j, 0, 4 * j + i4)], 0, pg[4 + i4]))
            for j in range(TJ - 1):
                ready = idx[(j, 1, 4 * j + 3)] + 1
                for g in outproj_groups(j):
                    fillers.append((NU, ready, g))
            fillers.sort(key=lambda f: (f[0], f[1]))
            NF = len(fillers)

            fi = 0
            ypairs = {}
            for s, (j, c, ii) in enumerate(sched):
                nii = 4 * j + 4
                while fi < NF and fillers[fi][0] <= s:
                    fillers[fi][2]()
                    fi += 1
                if ii == 0:
                    ypairs[c] = [
                        ps_y.tile([DH + 1, 512], F32, tag="y", name=f"y_{j}_{c}_{t}")
                        for t in range(2)
                    ]
                attn_unit(j, c, ii, nii, ypairs[c], [])
                if ii == nii - 1:
                    normalize(j, c, ypairs[c])
                target = (s + 1) * NF // NU
                while fi < min(target, NF) and fillers[fi][1] <= s:
                    fillers[fi][2]()
                    fi += 1
            while fi < NF:
                fillers[fi][2]()
                fi += 1
            for g in outproj_groups(TJ - 1, copy_on_act=True):
                g()
    nc.compile()
    return nc


def make_ident() -> np.ndarray:
    return np.eye(P, dtype=np.float32)


def make_maskb() -> np.ndarray:
    q = np.arange(P)[None, :]
    p = np.arange(P)[:, None]
    return np.ascontiguousarray((q >= p).astype(np.float32))


def qk_col_perm() -> np.ndarray:
    """Column permutation of wq/wk so the projection psum partitions land in
    the fp8 DoubleRow [32, 2] packing: matmul chunk c, out partition
    p = 32*h + p'  <->  feature f = 64*h + 32*c + p'."""
    perm = []
    for c in range(2):
        for jj in range(128):
            perm.append(64 * (jj // 32) + 32 * c + (jj % 32))
    return np.asarray(perm)


def shard_inputs(x, Wqkv, Wout):
    ident = make_ident().astype(BFNP)
    maskb = make_maskb().astype(BFNP)
    perm = qk_col_perm() if QK_FP8 else np.arange(F)
    in_maps = []
    for core in range(N_CORES):
        b, g = core // 4, core % 4
        sl = slice(F * g, F * (g + 1))
        wq = np.ascontiguousarray(Wqkv[sl, :].T[:, perm])
        wk = np.ascontiguousarray(Wqkv[D:][sl, :].T[:, perm])
        in_maps.append(
            {
                "xT": np.ascontiguousarray(x[b].T).astype(BFNP),
                "xT8": np.ascontiguousarray(x[b].T).astype(F8NP),
                "wqT": (wq * WQ_SCALE).astype(F8NP),
                "wkT": wk.astype(BFNP),
                "wvT": np.ascontiguousarray(Wqkv[2 * D:][sl, :].T).astype(BFNP),
                "woT": np.ascontiguousarray(Wout[:, sl].T).astype(BFNP),
                "ident": ident,
                "maskb": maskb,
            }
        )
    return in_maps


_NC_CACHE = None


def kernel(x, Wqkv, Wout):
    global _NC_CACHE
    x = np.asarray(x, dtype=np.float32)
    Wqkv = np.asarray(Wqkv, dtype=np.float32)
    Wout = np.asarray(Wout, dtype=np.float32)
    if _NC_CACHE is None:
        _NC_CACHE = build()
    nc = _NC_CACHE
    in_maps = shard_inputs(x, Wqkv, Wout)
    res = run_bass_kernel_spmd(nc, in_maps, core_ids=list(range(N_CORES)))
    outs = [res.results[c]["out"].astype(np.float32) for c in range(N_CORES)]
    return np.stack(
        [outs[0] + outs[1] + outs[2] + outs[3], outs[4] + outs[5] + outs[6] + outs[7]]
    )


# revision 31
# speedup vs baseline: 1.0419x; 1.0277x over previous
"""Multi-head causal self-attention (B=2, T=2048, D=1024, H=16, Dh=64) on 8 TRN2 cores.

Sharding: data-parallel over batch (2 groups of 4 cores), tensor-parallel over
heads within a group (4 heads/core). Each core computes its 4 heads'
QKV projection + causal flash attention + its slice of the output projection;
the host sums the 4 partial outputs per batch (bf16 partials, f32 sum).

Design (per core):
  - All matmuls bf16 (1 cyc/row at every N; fp32r pays 4x below N=256),
    except: (a) the score matmul - q/k are stored fp8e4m3 in a [32,2]
    DoubleRow packing, so S^T runs at 0.5 cyc/row (2x PE); (b) the
    q-projection itself - x and Wq ship as fp8 and contract DoubleRow
    (4 chunks of K=256), another 2x. Wq is pre-scaled by 32 on the host to
    lift it out of e4m3's subnormal range (compensated in the exp scale;
    the mask bias scales to -6144). The wq/wk columns are pre-permuted on
    the host so the projection psum lands directly in the packed S layout
    (the psum->sbuf copy is partition-preserving). k/v/out projections and
    PV stay bf16 - fp8 there fails the 2e-2 gate. Measured rel-err ~1.4e-2.
  - Attention runs transposed (S^T[tk, tq] blocks, 512-wide tq slices);
    softmax sums come free via a ones-column appended to V, so PV emits
    [y^T; sums] per head with no extra matmul cost (cost = moving dim only).
  - Causal mask is an additive bias matmul (identity lhsT x [-192 staircase]
    bf16) into the score psum on the mixed diagonal 128-block only; exp then
    yields e^-24 ~ 0 there. No vector-engine work on the exp->PV critical
    path. S matmuls/exps are column-restricted to the causal region;
    diagonal se tiles are dedicated per (pair, r) with their fully-masked
    left columns zeroed once at startup.
  - Head-pair S blocks land in one 2-bank psum tile => a single fused exp
    per (pair, ii) halves the ACT instruction count. ACT runs exp only;
    q/k/v psum copies are on DVE; softmax-sum broadcasts on gpsimd.
  - Emission is a deadline-driven schedule: every projection/out-projection
    matmul group becomes a filler with (deadline, ready) over a global unit
    order, flushed just-in-time or paced evenly, so the in-order PE queue
    never parks behind an exp-blocked score matmul. Out-projection drifts to
    the late (filler-starved) slices; the final slice normalizes in column
    chunks and alternates its psum copies between ACT and DVE to pipeline
    the tail stores.
  - Preload uses few large DMAs in consumption order (the DMA bus
    round-robins across queues, so small DMAs would let low-priority
    transfers steal bus turns from the critical first projection).
"""
import sys

import numpy as np

for _p in ("/opt/trn_rl_repo", "/root/.axon_site/_ro/trn_rl_repo"):
    if _p not in sys.path:
        try:
            import concourse  # noqa: F401
            break
        except ImportError:
            sys.path.append(_p)

import ml_dtypes  # noqa: E402
import concourse.bass as bass  # noqa: E402
import concourse.tile as tile  # noqa: E402
from concourse import bacc, mybir  # noqa: E402
from concourse.bass_utils import run_bass_kernel_spmd  # noqa: E402

P = 128
T = 2048
D = 1024
NH = 4          # heads per core
DH = 64
F = NH * DH     # per-core head features (256)
DC = D // P     # 8 contraction chunks
TJ = T // 512   # 4 tq slices
TC = T // P     # 16 tk chunks
N_CORES = 8
F32 = mybir.dt.float32
BF = mybir.dt.bfloat16
F8 = mybir.dt.float8e4
AF = mybir.ActivationFunctionType
DRM = mybir.MatmulPerfMode.DoubleRow
BFNP = ml_dtypes.bfloat16
F8NP = ml_dtypes.float8_e4m3

QK_FP8 = True  # q/k in fp8e4m3 + DoubleRow score matmul (2x PE on scores)

QK_DT = F8 if QK_FP8 else BF
WQ_SCALE = 32.0  # lifts Wq out of e4m3 subnormals; folded into exp scale
MASK_BIAS = -192.0 * WQ_SCALE  # exp(scale/WQ_SCALE * bias) = e^-24 ~ 4e-11


def build():
    nc = bacc.Bacc("TRN2", target_bir_lowering=False, debug=False, num_devices=N_CORES)
    xT = nc.dram_tensor("xT", [D, T], BF, kind="ExternalInput").ap()
    xT8 = nc.dram_tensor("xT8", [D, T], F8, kind="ExternalInput").ap()
    wqT = nc.dram_tensor("wqT", [D, F], F8, kind="ExternalInput").ap()
    wk8T = nc.dram_tensor("wk8T", [D // 2, F], F8, kind="ExternalInput").ap()
    wkT = nc.dram_tensor("wkT", [D // 2, F], BF, kind="ExternalInput").ap()
    wvT = nc.dram_tensor("wvT", [D, F], BF, kind="ExternalInput").ap()
    woT = nc.dram_tensor("woT", [F, D], BF, kind="ExternalInput").ap()
    ident = nc.dram_tensor("ident", [P, P], BF, kind="ExternalInput").ap()
    maskb = nc.dram_tensor("maskb", [P, P], BF, kind="ExternalInput").ap()
    out = nc.dram_tensor("out", [T, D], BF, kind="ExternalOutput").ap()

    scale = 1.0 / np.sqrt(DH) / (WQ_SCALE * WQ_SCALE)

    with tile.TileContext(nc) as tc:
        with (
            tc.tile_pool(name="weights", bufs=1) as wpool,
            tc.tile_pool(name="persist", bufs=1) as persist,
            tc.tile_pool(name="x", bufs=2) as xpool,
            tc.tile_pool(name="sexp", bufs=6) as sepool,
            tc.tile_pool(name="small", bufs=10) as small,
            tc.tile_pool(name="outsb", bufs=4) as opool,
            tc.tile_pool(name="ps_s", bufs=2, space="PSUM") as ps_s,
            tc.tile_pool(name="ps_y", bufs=2, space="PSUM") as ps_y,
            tc.tile_pool(name="ps_ao", bufs=2, space="PSUM") as ps_ao,
        ):
            wq_sb = wpool.tile([P, DC // 2, 2, F], F8)
            wk8_sb = wpool.tile([P, 2, 2, F], F8)
            wk_sb = wpool.tile([P, DC // 2, F], BF)
            wv_sb = wpool.tile([P, DC, F], BF)
            wo_sb = wpool.tile([P, 2, D], BF)
            mb_sb = wpool.tile([P, 1, P], BF)
            wq_r = wqT.rearrange("(o i p) f -> p o i f", p=P, i=2)
            wk8_r = wk8T.rearrange("(o i p) f -> p o i f", p=P, i=2)
            wk_r = wkT.rearrange("(o p) f -> p o f", p=P)
            wv_r = wvT.rearrange("(o p) f -> p o f", p=P)
            wo_r = woT.rearrange("(g p) e -> p g e", p=P)
            xT_r = xT.rearrange("(o p) t -> p o t", p=P)
            xT8_r = xT8.rearrange("(o i p) t -> p o i t", p=P, i=2)

            qT_sb = persist.tile([P, 2, T], QK_DT)
            kT_sb = persist.tile([P, 2, T], QK_DT)
            v_sb = persist.tile([P, NH, TC, DH + 1], BF)
            yT_sb = persist.tile([P, 2, T], BF)
            # dedicated diagonal se tiles per (pair c, r>=1): left cols hold
            # persistent zeros written once below.
            sediag = {
                (c, r): persist.tile([P, 2, 512], BF, name=f"sed_{c}_{r}")
                for c in range(2)
                for r in range(1, 4)
            }

            x_tiles = {}
            x8_tiles = {}

            def load_x(j):
                x_sb = xpool.tile([P, DC, 512], BF, tag="x", name=f"x_{j}")
                nc.sync.dma_start(x_sb[:], xT_r[:, :, 512 * j : 512 * (j + 1)])
                x_tiles[j] = x_sb
                x8_sb = xpool.tile([P, DC // 2, 2, 512], F8, tag="x8", name=f"x8_{j}")
                nc.sync.dma_start(x8_sb[:], xT8_r[:, :, :, 512 * j : 512 * (j + 1)])
                x8_tiles[j] = x8_sb

            # Preload in consumption order, few large DMAs: the DMA bus
            # round-robins across queues, so many small DMAs let low-priority
            # transfers steal bus turns from the critical first projection.
            x0_sb = xpool.tile([P, DC, 512], BF, tag="x", name="x_0")
            x_tiles[0] = x0_sb
            x80_sb = xpool.tile([P, DC // 2, 2, 512], F8, tag="x8", name="x8_0")
            x8_tiles[0] = x80_sb
            nc.sync.dma_start(wq_sb[:], wq_r[:])
            nc.sync.dma_start(x80_sb[:, 0:2], xT8_r[:, 0:2, :, 0:512])
            nc.sync.dma_start(x80_sb[:, 2:4], xT8_r[:, 2:4, :, 0:512])
            nc.sync.dma_start(wk8_sb[:], wk8_r[:])
            nc.sync.dma_start(wk_sb[:], wk_r[:])
            nc.sync.dma_start(x0_sb[:, 0:4], xT_r[:, 0:4, 0:512])
            nc.sync.dma_start(x0_sb[:, 4:8], xT_r[:, 4:8, 0:512])
            nc.sync.dma_start(wv_sb[:], wv_r[:])
            nc.sync.dma_start(mb_sb[:, 0], maskb)
            load_x(1)
            nc.sync.dma_start(wo_sb[:], wo_r[:])

            # V's softmax-sum ones column + persistent zeros in the
            # fully-masked left region of diagonal se tiles.
            nc.gpsimd.memset(v_sb[:, :, :, DH : DH + 1], 1.0)
            for (c, r), t_ in sediag.items():
                nc.gpsimd.memset(t_[:, :, 0 : 128 * r], 0.0)

            def proj_groups(j):
                jsl = slice(512 * j, 512 * (j + 1))
                x_sb = x_tiles[j]
                groups = []
                x8_sb = x8_tiles[j]
                for c in range(2):
                    def g(c=c):
                        pt = ps_ao.tile([P, 512], F32, tag="ao")
                        for o in range(DC // 2):
                            nc.tensor.matmul(
                                pt[:],
                                wq_sb[:, o, :, 128 * c : 128 * (c + 1)],
                                x8_sb[:, o, :, :],
                                start=(o == 0),
                                stop=(o == DC // 2 - 1),
                                perf_mode=DRM,
                            )
                        nc.vector.tensor_copy(qT_sb[:, c, jsl], pt[:])
                    groups.append(g)
                for c in range(2):
                    def g(c=c):
                        pt = ps_ao.tile([P, 512], F32, tag="ao")
                        for o in range(2):
                            nc.tensor.matmul(
                                pt[:],
                                wk8_sb[:, o, :, 128 * c : 128 * (c + 1)],
                                x8_sb[:, o, :, :],
                                start=(o == 0),
                                stop=False,
                                perf_mode=DRM,
                            )
                        for o in range(4, DC):
                            nc.tensor.matmul(
                                pt[:],
                                wk_sb[:, o - 4, 128 * c : 128 * (c + 1)],
                                x_sb[:, o, :],
                                start=False,
                                stop=(o == DC - 1),
                            )
                        nc.vector.tensor_copy(kT_sb[:, c, jsl], pt[:])
                    groups.append(g)
                for i in range(4):
                    def g(i=i):
                        pt = ps_ao.tile([P, 512], F32, tag="ao")
                        for o in range(DC):
                            nc.tensor.matmul(
                                pt[:, :F],
                                x_sb[:, o, 128 * i : 128 * (i + 1)],
                                wv_sb[:, o, :],
                                start=(o == 0),
                                stop=(o == DC - 1),
                            )
                        nc.vector.tensor_copy(
                            v_sb[:, :, 4 * j + i, 0:DH],
                            pt[:, :F].rearrange("p (h d) -> p h d", h=NH),
                        )
                    groups.append(g)
                return groups

            def outproj_groups(j, copy_on_act=False):
                groups = []
                for tb in range(4 * j, 4 * (j + 1)):
                    for eb in range(2):
                        def g(tb=tb, eb=eb):
                            pt = ps_ao.tile([P, 512], F32, tag="ao")
                            for g2 in range(2):
                                nc.tensor.matmul(
                                    pt[:],
                                    yT_sb[:, g2, 128 * tb : 128 * (tb + 1)],
                                    wo_sb[:, g2, 512 * eb : 512 * (eb + 1)],
                                    start=(g2 == 0),
                                    stop=(g2 == 1),
                                )
                            osb = opool.tile([P, 512], BF, tag="osb")
                            if copy_on_act and (tb + eb) % 2 == 0:
                                nc.scalar.copy(osb[:], pt[:])
                            else:
                                nc.vector.tensor_copy(osb[:], pt[:])
                            nc.sync.dma_start(
                                out[128 * tb : 128 * (tb + 1), 512 * eb : 512 * (eb + 1)],
                                osb[:],
                            )
                        groups.append(g)
                return groups

            def attn_unit(j, c, ii, nii, ypair, mid):
                r = ii - 4 * j
                col0 = 128 * r if r > 0 else 0
                qsl = slice(512 * j + col0, 512 * (j + 1))
                diag = r >= 0
                sps = ps_s.tile([P, 2, 512], F32, tag="s")
                for t in range(2):
                    if QK_FP8:
                        h = 2 * c + t
                        nc.tensor.matmul(
                            sps[:, t, col0:],
                            kT_sb[32 * h : 32 * h + 32, :, 128 * ii : 128 * (ii + 1)],
                            qT_sb[32 * h : 32 * h + 32, :, qsl],
                            start=True,
                            stop=True,
                            perf_mode=DRM,
                            tile_position=(32 * h, 0),
                        )
                    else:
                        hp = 64 * t
                        nc.tensor.matmul(
                            sps[:, t, col0:],
                            kT_sb[hp : hp + DH, c, 128 * ii : 128 * (ii + 1)],
                            qT_sb[hp : hp + DH, c, qsl],
                            start=True,
                            stop=True,
                        )

                se = (
                    sediag[(c, r)]
                    if r >= 1
                    else sepool.tile([P, 2, 512], BF, tag="se")
                )
                nc.scalar.activation(
                    se[:, :, col0:], sps[:, :, col0:], AF.Exp, scale=scale
                )
                if diag:
                    # zero the strictly-upper part of the mixed 128-block
                    nc.vector.tensor_mul(
                        se[:, :, col0 : col0 + 128],
                        se[:, :, col0 : col0 + 128],
                        mb_sb[:, 0:1, :].to_broadcast([P, 2, P]),
                    )
                for f in mid:
                    f()
                for t in range(2):
                    nc.tensor.matmul(
                        ypair[t][:],
                        v_sb[:, 2 * c + t, ii, :],
                        se[:, t, :],
                        start=(ii == 0),
                        stop=(ii == nii - 1),
                    )

            def normalize(j, c, ypair):
                # All recips first (DVE), broadcasts next (Pool, overlaps the
                # recips), muls last — so the in-order DVE queue never parks
                # behind a Pool broadcast. Chunked on the last slice so
                # outproj(3) tb-group tb starts as soon as its yT chunk lands.
                nchunk = 2 if j == TJ - 1 else 1
                w = 512 // nchunk
                bsbs = {}
                for q4 in range(nchunk):
                    for t in range(2):
                        qs = slice(w * q4, w * (q4 + 1))
                        rec = small.tile([1, w], F32, tag=f"rec{nchunk}")
                        nc.vector.reciprocal(rec[:], ypair[t][DH : DH + 1, qs])
                        bsb = small.tile([DH, w], F32, tag=f"bsb{nchunk}")
                        nc.gpsimd.partition_broadcast(bsb[:], rec[:])
                        bsbs[(t, q4)] = bsb
                for q4 in range(nchunk):
                    for t in range(2):
                        qs = slice(w * q4, w * (q4 + 1))
                        ys = slice(512 * j + w * q4, 512 * j + w * (q4 + 1))
                        nc.vector.tensor_mul(
                            yT_sb[64 * t : 64 * t + DH, c, ys],
                            ypair[t][0:DH, qs],
                            bsbs[(t, q4)],
                        )

            # Remaining x slices: queued behind the preload on the same DMA
            # ring, arriving well before their projections need them.
            load_x(2)
            load_x(3)

            # ---- deadline-driven emission schedule ----
            # One global unit order; every proj/outproj matmul group becomes
            # a filler with (deadline unit, ready unit). Deadline fillers are
            # flushed right before their unit; the rest pace out evenly so
            # the in-order PE queue always has independent work between
            # exp-blocked attention steps. proj(0) interleaves into the j=0
            # units, so the ACT exp pipeline starts ~14us earlier.
            sched = []
            for j in range(TJ):
                for c in range(2):
                    for ii in range(4 * j + 4):
                        sched.append((j, c, ii))
            idx = {u: s for s, u in enumerate(sched)}
            NU = len(sched)

            fillers = []  # (deadline, ready, fn)
            for j in range(TJ):
                pg = proj_groups(j)  # [q-c0, q-c1, k-c0, k-c1, v0..v3] halves
                for c in range(2):
                    # fp8 layout: dim1 of qT/kT is the dh-tile, so every S
                    # matmul reads BOTH projection chunks => both must land
                    # before the slice's first unit.
                    d = idx[(j, 0, 0)] if QK_FP8 else idx[(j, c, 0)]
                    fillers.append((d, 0, pg[c]))
                    fillers.append((d, 0, pg[2 + c]))
                for i4 in range(4):
                    fillers.append((idx[(j, 0, 4 * j + i4)], 0, pg[4 + i4]))
            for j in range(TJ - 1):
                ready = idx[(j, 1, 4 * j + 3)] + 1
                for g in outproj_groups(j):
                    fillers.append((NU, ready, g))
            fillers.sort(key=lambda f: (f[0], f[1]))
            NF = len(fillers)

            fi = 0
            ypairs = {}
            for s, (j, c, ii) in enumerate(sched):
                nii = 4 * j + 4
                while fi < NF and fillers[fi][0] <= s:
                    fillers[fi][2]()
                    fi += 1
                if ii == 0:
                    ypairs[c] = [
                        ps_y.tile([DH + 1, 512], F32, tag="y", name=f"y_{j}_{c}_{t}")
                        for t in range(2)
                    ]
                attn_unit(j, c, ii, nii, ypairs[c], [])
                if ii == nii - 1:
                    normalize(j, c, ypairs[c])
                target = (s + 1) * NF // NU
                while fi < min(target, NF) and fillers[fi][1] <= s:
                    fillers[fi][2]()
                    fi += 1
            while fi < NF:
                fillers[fi][2]()
                fi += 1
            for g in outproj_groups(TJ - 1, copy_on_act=True):
                g()
    nc.compile()
    return nc


def make_ident() -> np.ndarray:
    return np.eye(P, dtype=np.float32)


def make_maskb() -> np.ndarray:
    q = np.arange(P)[None, :]
    p = np.arange(P)[:, None]
    return np.ascontiguousarray((q >= p).astype(np.float32))


def qk_col_perm() -> np.ndarray:
    """Column permutation of wq/wk so the projection psum partitions land in
    the fp8 DoubleRow [32, 2] packing: matmul chunk c, out partition
    p = 32*h + p'  <->  feature f = 64*h + 32*c + p'."""
    perm = []
    for c in range(2):
        for jj in range(128):
            perm.append(64 * (jj // 32) + 32 * c + (jj % 32))
    return np.asarray(perm)


def shard_inputs(x, Wqkv, Wout):
    ident = make_ident().astype(BFNP)
    maskb = make_maskb().astype(BFNP)
    perm = qk_col_perm() if QK_FP8 else np.arange(F)
    in_maps = []
    for core in range(N_CORES):
        b, g = core // 4, core % 4
        sl = slice(F * g, F * (g + 1))
        wq = np.ascontiguousarray(Wqkv[sl, :].T[:, perm])
        wk = np.ascontiguousarray(Wqkv[D:][sl, :].T[:, perm])
        in_maps.append(
            {
                "xT": np.ascontiguousarray(x[b].T).astype(BFNP),
                "xT8": np.ascontiguousarray(x[b].T).astype(F8NP),
                "wqT": (wq * WQ_SCALE).astype(F8NP),
                "wk8T": (wk[:D // 2] * WQ_SCALE).astype(F8NP),
                "wkT": (wk[D // 2:] * WQ_SCALE).astype(BFNP),
                "wvT": np.ascontiguousarray(Wqkv[2 * D:][sl, :].T).astype(BFNP),
                "woT": np.ascontiguousarray(Wout[:, sl].T).astype(BFNP),
                "ident": ident,
                "maskb": maskb,
            }
        )
    return in_maps


_NC_CACHE = None


def kernel(x, Wqkv, Wout):
    global _NC_CACHE
    x = np.asarray(x, dtype=np.float32)
    Wqkv = np.asarray(Wqkv, dtype=np.float32)
    Wout = np.asarray(Wout, dtype=np.float32)
    if _NC_CACHE is None:
        _NC_CACHE = build()
    nc = _NC_CACHE
    in_maps = shard_inputs(x, Wqkv, Wout)
    res = run_bass_kernel_spmd(nc, in_maps, core_ids=list(range(N_CORES)))
    outs = [res.results[c]["out"].astype(np.float32) for c in range(N_CORES)]
    return np.stack(
        [outs[0] + outs[1] + outs[2] + outs[3], outs[4] + outs[5] + outs[6] + outs[7]]
    )


# revision 34
# speedup vs baseline: 1.0523x; 1.0100x over previous
"""Multi-head causal self-attention (B=2, T=2048, D=1024, H=16, Dh=64) on 8 TRN2 cores.

Sharding: data-parallel over batch (2 groups of 4 cores), tensor-parallel over
heads within a group (4 heads/core). Each core computes its 4 heads'
QKV projection + causal flash attention + its slice of the output projection;
the host sums the 4 partial outputs per batch (bf16 partials, f32 sum).

Design (per core):
  - All matmuls bf16 (1 cyc/row at every N; fp32r pays 4x below N=256),
    except: (a) the score matmul - q/k are stored fp8e4m3 in a [32,2]
    DoubleRow packing, so S^T runs at 0.5 cyc/row (2x PE); (b) the
    q-projection itself - x and Wq ship as fp8 and contract DoubleRow
    (4 chunks of K=256), another 2x. Wq is pre-scaled by 32 on the host to
    lift it out of e4m3's subnormal range (compensated in the exp scale;
    the mask bias scales to -6144). The wq/wk columns are pre-permuted on
    the host so the projection psum lands directly in the packed S layout
    (the psum->sbuf copy is partition-preserving). k/v/out projections and
    PV stay bf16 - fp8 there fails the 2e-2 gate. Measured rel-err ~1.4e-2.
  - Attention runs transposed (S^T[tk, tq] blocks, 512-wide tq slices);
    softmax sums come free via a ones-column appended to V, so PV emits
    [y^T; sums] per head with no extra matmul cost (cost = moving dim only).
  - Causal mask is an additive bias matmul (identity lhsT x [-192 staircase]
    bf16) into the score psum on the mixed diagonal 128-block only; exp then
    yields e^-24 ~ 0 there. No vector-engine work on the exp->PV critical
    path. S matmuls/exps are column-restricted to the causal region;
    diagonal se tiles are dedicated per (pair, r) with their fully-masked
    left columns zeroed once at startup.
  - Head-pair S blocks land in one 2-bank psum tile => a single fused exp
    per (pair, ii) halves the ACT instruction count. ACT runs exp only;
    q/k/v psum copies are on DVE; softmax-sum broadcasts on gpsimd.
  - Emission is a deadline-driven schedule: every projection/out-projection
    matmul group becomes a filler with (deadline, ready) over a global unit
    order, flushed just-in-time or paced evenly, so the in-order PE queue
    never parks behind an exp-blocked score matmul. Out-projection drifts to
    the late (filler-starved) slices; the final slice normalizes in column
    chunks and alternates its psum copies between ACT and DVE to pipeline
    the tail stores.
  - Preload uses few large DMAs in consumption order (the DMA bus
    round-robins across queues, so small DMAs would let low-priority
    transfers steal bus turns from the critical first projection).
"""
import sys

import numpy as np

for _p in ("/opt/trn_rl_repo", "/root/.axon_site/_ro/trn_rl_repo"):
    if _p not in sys.path:
        try:
            import concourse  # noqa: F401
            break
        except ImportError:
            sys.path.append(_p)

import ml_dtypes  # noqa: E402
import concourse.bass as bass  # noqa: E402
import concourse.tile as tile  # noqa: E402
from concourse import bacc, mybir  # noqa: E402
from concourse.bass_utils import run_bass_kernel_spmd  # noqa: E402

P = 128
T = 2048
D = 1024
NH = 4          # heads per core
DH = 64
F = NH * DH     # per-core head features (256)
DC = D // P     # 8 contraction chunks
TJ = T // 512   # 4 tq slices
TC = T // P     # 16 tk chunks
N_CORES = 8
F32 = mybir.dt.float32
BF = mybir.dt.bfloat16
F8 = mybir.dt.float8e4
AF = mybir.ActivationFunctionType
DRM = mybir.MatmulPerfMode.DoubleRow
BFNP = ml_dtypes.bfloat16
F8NP = ml_dtypes.float8_e4m3

QK_FP8 = True  # q/k in fp8e4m3 + DoubleRow score matmul (2x PE on scores)

QK_DT = F8 if QK_FP8 else BF
WQ_SCALE = 32.0  # lifts Wq out of e4m3 subnormals; folded into exp scale
MASK_BIAS = -192.0 * WQ_SCALE  # exp(scale/WQ_SCALE * bias) = e^-24 ~ 4e-11


def build():
    nc = bacc.Bacc("TRN2", target_bir_lowering=False, debug=False, num_devices=N_CORES)
    xT = nc.dram_tensor("xT", [D, T], BF, kind="ExternalInput").ap()
    xT8 = nc.dram_tensor("xT8", [D, T], F8, kind="ExternalInput").ap()
    wqT = nc.dram_tensor("wqT", [D, F], F8, kind="ExternalInput").ap()
    wk8T = nc.dram_tensor("wk8T", [D // 2, F], F8, kind="ExternalInput").ap()
    wkT = nc.dram_tensor("wkT", [D // 2, F], BF, kind="ExternalInput").ap()
    wvT = nc.dram_tensor("wvT", [D, F], BF, kind="ExternalInput").ap()
    woT = nc.dram_tensor("woT", [F, D], BF, kind="ExternalInput").ap()
    ident = nc.dram_tensor("ident", [P, P], BF, kind="ExternalInput").ap()
    maskb = nc.dram_tensor("maskb", [P, P], BF, kind="ExternalInput").ap()
    out = nc.dram_tensor("out", [T, D], BF, kind="ExternalOutput").ap()

    scale = 1.0 / np.sqrt(DH) / (WQ_SCALE * WQ_SCALE)

    with tile.TileContext(nc) as tc:
        with (
            tc.tile_pool(name="weights", bufs=1) as wpool,
            tc.tile_pool(name="persist", bufs=1) as persist,
            tc.tile_pool(name="x", bufs=2) as xpool,
            tc.tile_pool(name="sexp", bufs=6) as sepool,
            tc.tile_pool(name="small", bufs=10) as small,
            tc.tile_pool(name="outsb", bufs=4) as opool,
            tc.tile_pool(name="ps_s", bufs=2, space="PSUM") as ps_s,
            tc.tile_pool(name="ps_y", bufs=2, space="PSUM") as ps_y,
            tc.tile_pool(name="ps_ao", bufs=2, space="PSUM") as ps_ao,
        ):
            wq_sb = wpool.tile([P, DC // 2, 2, F], F8)
            wk8_sb = wpool.tile([P, 2, 2, F], F8)
            wk_sb = wpool.tile([P, DC // 2, F], BF)
            wv_sb = wpool.tile([P, DC, F], BF)
            wo_sb = wpool.tile([P, 2, D], BF)
            mb_sb = wpool.tile([P, 1, P], BF)
            wq_r = wqT.rearrange("(o i p) f -> p o i f", p=P, i=2)
            wk8_r = wk8T.rearrange("(o i p) f -> p o i f", p=P, i=2)
            wk_r = wkT.rearrange("(o p) f -> p o f", p=P)
            wv_r = wvT.rearrange("(o p) f -> p o f", p=P)
            wo_r = woT.rearrange("(g p) e -> p g e", p=P)
            xT_r = xT.rearrange("(o p) t -> p o t", p=P)
            xT8_r = xT8.rearrange("(o i p) t -> p o i t", p=P, i=2)

            qT_sb = persist.tile([P, 2, T], QK_DT)
            kT_sb = persist.tile([P, 2, T], QK_DT)
            v_sb = persist.tile([P, NH, TC, DH + 1], BF)
            yT_sb = persist.tile([P, 2, T], BF)
            # dedicated diagonal se tiles per (pair c, r>=1): left cols hold
            # persistent zeros written once below.
            sediag = {
                (c, r): persist.tile([P, 2, 512], BF, name=f"sed_{c}_{r}")
                for c in range(2)
                for r in range(1, 4)
            }

            x_tiles = {}
            x8_tiles = {}

            def load_x(j):
                x_sb = xpool.tile([P, DC, 512], BF, tag="x", name=f"x_{j}")
                nc.sync.dma_start(x_sb[:], xT_r[:, :, 512 * j : 512 * (j + 1)])
                x_tiles[j] = x_sb
                x8_sb = xpool.tile([P, DC // 2, 2, 512], F8, tag="x8", name=f"x8_{j}")
                nc.sync.dma_start(x8_sb[:], xT8_r[:, :, :, 512 * j : 512 * (j + 1)])
                x8_tiles[j] = x8_sb

            # Preload in consumption order, few large DMAs: the DMA bus
            # round-robins across queues, so many small DMAs let low-priority
            # transfers steal bus turns from the critical first projection.
            x0_sb = xpool.tile([P, DC, 512], BF, tag="x", name="x_0")
            x_tiles[0] = x0_sb
            x80_sb = xpool.tile([P, DC // 2, 2, 512], F8, tag="x8", name="x8_0")
            x8_tiles[0] = x80_sb
            nc.sync.dma_start(wq_sb[:], wq_r[:])
            nc.sync.dma_start(x80_sb[:, 0:2], xT8_r[:, 0:2, :, 0:512])
            nc.sync.dma_start(x80_sb[:, 2:4], xT8_r[:, 2:4, :, 0:512])
            nc.sync.dma_start(wk8_sb[:], wk8_r[:])
            nc.sync.dma_start(wk_sb[:], wk_r[:])
            nc.sync.dma_start(x0_sb[:, 0:4], xT_r[:, 0:4, 0:512])
            nc.sync.dma_start(x0_sb[:, 4:8], xT_r[:, 4:8, 0:512])
            nc.sync.dma_start(wv_sb[:], wv_r[:])
            nc.sync.dma_start(mb_sb[:, 0], maskb)
            load_x(1)
            nc.sync.dma_start(wo_sb[:], wo_r[:])

            # V's softmax-sum ones column + persistent zeros in the
            # fully-masked left region of diagonal se tiles.
            nc.gpsimd.memset(v_sb[:, :, :, DH : DH + 1], 1.0)
            for (c, r), t_ in sediag.items():
                nc.gpsimd.memset(t_[:, :, 0 : 128 * r], 0.0)

            def proj_groups(j):
                jsl = slice(512 * j, 512 * (j + 1))
                x_sb = x_tiles[j]
                groups = []
                x8_sb = x8_tiles[j]
                for c in range(2):
                    def g(c=c):
                        pt = ps_ao.tile([P, 512], F32, tag="ao")
                        for o in range(DC // 2):
                            nc.tensor.matmul(
                                pt[:],
                                wq_sb[:, o, :, 128 * c : 128 * (c + 1)],
                                x8_sb[:, o, :, :],
                                start=(o == 0),
                                stop=(o == DC // 2 - 1),
                                perf_mode=DRM,
                            )
                        nc.vector.tensor_copy(qT_sb[:, c, jsl], pt[:])
                    groups.append(g)
                for c in range(2):
                    def g(c=c):
                        pt = ps_ao.tile([P, 512], F32, tag="ao")
                        for o in range(2):
                            nc.tensor.matmul(
                                pt[:],
                                wk8_sb[:, o, :, 128 * c : 128 * (c + 1)],
                                x8_sb[:, o, :, :],
                                start=(o == 0),
                                stop=False,
                                perf_mode=DRM,
                            )
                        for o in range(4, DC):
                            nc.tensor.matmul(
                                pt[:],
                                wk_sb[:, o - 4, 128 * c : 128 * (c + 1)],
                                x_sb[:, o, :],
                                start=False,
                                stop=(o == DC - 1),
                            )
                        nc.vector.tensor_copy(kT_sb[:, c, jsl], pt[:])
                    groups.append(g)
                for i in range(4):
                    def g(i=i):
                        pt = ps_ao.tile([P, 512], F32, tag="ao")
                        for o in range(DC):
                            nc.tensor.matmul(
                                pt[:, :F],
                                x_sb[:, o, 128 * i : 128 * (i + 1)],
                                wv_sb[:, o, :],
                                start=(o == 0),
                                stop=(o == DC - 1),
                            )
                        nc.vector.tensor_copy(
                            v_sb[:, :, 4 * j + i, 0:DH],
                            pt[:, :F].rearrange("p (h d) -> p h d", h=NH),
                        )
                    groups.append(g)
                return groups

            def outproj_groups(j, copy_on_act=False):
                groups = []
                for tb in range(4 * j, 4 * (j + 1)):
                    for eb in range(2):
                        def g(tb=tb, eb=eb):
                            pt = ps_ao.tile([P, 512], F32, tag="ao")
                            for g2 in range(2):
                                nc.tensor.matmul(
                                    pt[:],
                                    yT_sb[:, g2, 128 * tb : 128 * (tb + 1)],
                                    wo_sb[:, g2, 512 * eb : 512 * (eb + 1)],
                                    start=(g2 == 0),
                                    stop=(g2 == 1),
                                )
                            osb = opool.tile([P, 512], BF, tag="osb")
                            if copy_on_act and (tb + eb) % 2 == 0:
                                nc.scalar.copy(osb[:], pt[:])
                            else:
                                nc.vector.tensor_copy(osb[:], pt[:])
                            nc.sync.dma_start(
                                out[128 * tb : 128 * (tb + 1), 512 * eb : 512 * (eb + 1)],
                                osb[:],
                            )
                        groups.append(g)
                return groups

            def attn_unit(j, c, ii, nii, ypair, mid):
                r = ii - 4 * j
                col0 = 128 * r if r > 0 else 0
                qsl = slice(512 * j + col0, 512 * (j + 1))
                diag = r >= 0
                sps = ps_s.tile([P, 2, 512], F32, tag="s")
                for t in range(2):
                    if QK_FP8:
                        h = 2 * c + t
                        nc.tensor.matmul(
                            sps[:, t, col0:],
                            kT_sb[32 * h : 32 * h + 32, :, 128 * ii : 128 * (ii + 1)],
                            qT_sb[32 * h : 32 * h + 32, :, qsl],
                            start=True,
                            stop=True,
                            perf_mode=DRM,
                            tile_position=(32 * h, 0),
                        )
                    else:
                        hp = 64 * t
                        nc.tensor.matmul(
                            sps[:, t, col0:],
                            kT_sb[hp : hp + DH, c, 128 * ii : 128 * (ii + 1)],
                            qT_sb[hp : hp + DH, c, qsl],
                            start=True,
                            stop=True,
                        )

                se = (
                    sediag[(c, r)]
                    if r >= 1
                    else sepool.tile([P, 2, 512], BF, tag="se")
                )
                nc.scalar.activation(
                    se[:, :, col0:], sps[:, :, col0:], AF.Exp, scale=scale
                )
                if diag:
                    # zero the strictly-upper part of the mixed 128-block
                    nc.vector.tensor_mul(
                        se[:, :, col0 : col0 + 128],
                        se[:, :, col0 : col0 + 128],
                        mb_sb[:, 0:1, :].to_broadcast([P, 2, P]),
                    )
                for f in mid:
                    f()
                for t in range(2):
                    nc.tensor.matmul(
                        ypair[t][:],
                        v_sb[:, 2 * c + t, ii, :],
                        se[:, t, :],
                        start=(ii == 0),
                        stop=(ii == nii - 1),
                    )

            def normalize(j, c, ypair):
                # All recips first (DVE), broadcasts next (Pool, overlaps the
                # recips), muls last — so the in-order DVE queue never parks
                # behind a Pool broadcast. Chunked on the last slice so
                # outproj(3) tb-group tb starts as soon as its yT chunk lands.
                nchunk = 2 if j == TJ - 1 else 1
                w = 512 // nchunk
                bsbs = {}
                for q4 in range(nchunk):
                    for t in range(2):
                        qs = slice(w * q4, w * (q4 + 1))
                        rec = small.tile([1, w], F32, tag=f"rec{nchunk}")
                        nc.vector.reciprocal(rec[:], ypair[t][DH : DH + 1, qs])
                        bsb = small.tile([DH, w], F32, tag=f"bsb{nchunk}")
                        nc.gpsimd.partition_broadcast(bsb[:], rec[:])
                        bsbs[(t, q4)] = bsb
                for q4 in range(nchunk):
                    for t in range(2):
                        qs = slice(w * q4, w * (q4 + 1))
                        ys = slice(512 * j + w * q4, 512 * j + w * (q4 + 1))
                        nc.vector.tensor_mul(
                            yT_sb[64 * t : 64 * t + DH, c, ys],
                            ypair[t][0:DH, qs],
                            bsbs[(t, q4)],
                        )

            # Remaining x slices: queued behind the preload on the same DMA
            # ring, arriving well before their projections need them.
            load_x(2)
            load_x(3)

            # ---- deadline-driven emission schedule ----
            # One global unit order; every proj/outproj matmul group becomes
            # a filler with (deadline unit, ready unit). Deadline fillers are
            # flushed right before their unit; the rest pace out evenly so
            # the in-order PE queue always has independent work between
            # exp-blocked attention steps. proj(0) interleaves into the j=0
            # units, so the ACT exp pipeline starts ~14us earlier.
            sched = []
            for j in range(TJ):
                for c in range(2):
                    for ii in range(4 * j + 4):
                        sched.append((j, c, ii))
            idx = {u: s for s, u in enumerate(sched)}
            NU = len(sched)

            fillers = []  # (deadline, ready, fn, pe_cost)
            QC, KC, VC, OC = 1024, 2560, 4096, 1024  # PE cycles per group
            for j in range(TJ):
                pg = proj_groups(j)  # [q-c0, q-c1, k-c0, k-c1, v0..v3]
                for c in range(2):
                    # fp8 layout: dim1 of qT/kT is the dh-tile, so every S
                    # matmul reads BOTH projection chunks => both must land
                    # before the slice's first unit.
                    d = idx[(j, 0, 0)] if QK_FP8 else idx[(j, c, 0)]
                    fillers.append((d, 0, pg[c], QC))
                    fillers.append((d, 0, pg[2 + c], KC))
                for i4 in range(4):
                    fillers.append((idx[(j, 0, 4 * j + i4)], 0, pg[4 + i4], VC))
            for j in range(TJ - 1):
                ready = idx[(j, 1, 4 * j + 3)] + 1
                for g in outproj_groups(j):
                    fillers.append((NU, ready, g, OC))
            fillers.sort(key=lambda f: (f[0], f[1]))
            NF = len(fillers)
            total_cost = sum(f[3] for f in fillers)

            fi = 0
            done_cost = 0
            ypairs = {}
            for s, (j, c, ii) in enumerate(sched):
                nii = 4 * j + 4
                while fi < NF and fillers[fi][0] <= s:
                    done_cost += fillers[fi][3]
                    fillers[fi][2]()
                    fi += 1
                if ii == 0:
                    ypairs[c] = [
                        ps_y.tile([DH + 1, 512], F32, tag="y", name=f"y_{j}_{c}_{t}")
                        for t in range(2)
                    ]
                attn_unit(j, c, ii, nii, ypairs[c], [])
                if ii == nii - 1:
                    normalize(j, c, ypairs[c])
                # pace by PE-time, not group count: a v-group is 4x a q-group
                target = (s + 1) * total_cost // NU
                while fi < NF and done_cost < target and fillers[fi][1] <= s:
                    done_cost += fillers[fi][3]
                    fillers[fi][2]()
                    fi += 1
            while fi < NF:
                fillers[fi][2]()
                fi += 1
            for g in outproj_groups(TJ - 1, copy_on_act=True):
                g()
    nc.compile()
    return nc


def make_ident() -> np.ndarray:
    return np.eye(P, dtype=np.float32)


def make_maskb() -> np.ndarray:
    q = np.arange(P)[None, :]
    p = np.arange(P)[:, None]
    return np.ascontiguousarray((q >= p).astype(np.float32))


def qk_col_perm() -> np.ndarray:
    """Column permutation of wq/wk so the projection psum partitions land in
    the fp8 DoubleRow [32, 2] packing: matmul chunk c, out partition
    p = 32*h + p'  <->  feature f = 64*h + 32*c + p'."""
    perm = []
    for c in range(2):
        for jj in range(128):
            perm.append(64 * (jj // 32) + 32 * c + (jj % 32))
    return np.asarray(perm)


def shard_inputs(x, Wqkv, Wout):
    ident = make_ident().astype(BFNP)
    maskb = make_maskb().astype(BFNP)
    perm = qk_col_perm() if QK_FP8 else np.arange(F)
    in_maps = []
    for core in range(N_CORES):
        b, g = core // 4, core % 4
        sl = slice(F * g, F * (g + 1))
        wq = np.ascontiguousarray(Wqkv[sl, :].T[:, perm])
        wk = np.ascontiguousarray(Wqkv[D:][sl, :].T[:, perm])
        in_maps.append(
            {
                "xT": np.ascontiguousarray(x[b].T).astype(BFNP),
                "xT8": np.ascontiguousarray(x[b].T).astype(F8NP),
                "wqT": (wq * WQ_SCALE).astype(F8NP),
                "wk8T": (wk[:D // 2] * WQ_SCALE).astype(F8NP),
                "wkT": (wk[D // 2:] * WQ_SCALE).astype(BFNP),
                "wvT": np.ascontiguousarray(Wqkv[2 * D:][sl, :].T).astype(BFNP),
                "woT": np.ascontiguousarray(Wout[:, sl].T).astype(BFNP),
                "ident": ident,
                "maskb": maskb,
            }
        )
    return in_maps


_NC_CACHE = None


def kernel(x, Wqkv, Wout):
    global _NC_CACHE
    x = np.asarray(x, dtype=np.float32)
    Wqkv = np.asarray(Wqkv, dtype=np.float32)
    Wout = np.asarray(Wout, dtype=np.float32)
    if _NC_CACHE is None:
        _NC_CACHE = build()
    nc = _NC_CACHE
    in_maps = shard_inputs(x, Wqkv, Wout)
    res = run_bass_kernel_spmd(nc, in_maps, core_ids=list(range(N_CORES)))
    outs = [res.results[c]["out"].astype(np.float32) for c in range(N_CORES)]
    return np.stack(
        [outs[0] + outs[1] + outs[2] + outs[3], outs[4] + outs[5] + outs[6] + outs[7]]
    )


# revision 41
# speedup vs baseline: 1.0569x; 1.0044x over previous
"""Multi-head causal self-attention (B=2, T=2048, D=1024, H=16, Dh=64) on 8 TRN2 cores.

Sharding: data-parallel over batch (2 groups of 4 cores), tensor-parallel over
heads within a group (4 heads/core). Each core computes its 4 heads'
QKV projection + causal flash attention + its slice of the output projection;
the host sums the 4 partial outputs per batch (bf16 partials, f32 sum).

Design (per core):
  - All matmuls bf16 (1 cyc/row at every N; fp32r pays 4x below N=256),
    except: (a) the score matmul - q/k are stored fp8e4m3 in a [32,2]
    DoubleRow packing, so S^T runs at 0.5 cyc/row (2x PE); (b) the
    q-projection itself - x and Wq ship as fp8 and contract DoubleRow
    (4 chunks of K=256), another 2x. Wq is pre-scaled by 32 on the host to
    lift it out of e4m3's subnormal range (compensated in the exp scale;
    the mask bias scales to -6144). The wq/wk columns are pre-permuted on
    the host so the projection psum lands directly in the packed S layout
    (the psum->sbuf copy is partition-preserving). k/v/out projections and
    PV stay bf16 - fp8 there fails the 2e-2 gate. Measured rel-err ~1.4e-2.
  - Attention runs transposed (S^T[tk, tq] blocks, 512-wide tq slices);
    softmax sums come free via a ones-column appended to V, so PV emits
    [y^T; sums] per head with no extra matmul cost (cost = moving dim only).
  - Causal mask is an additive bias matmul (identity lhsT x [-192 staircase]
    bf16) into the score psum on the mixed diagonal 128-block only; exp then
    yields e^-24 ~ 0 there. No vector-engine work on the exp->PV critical
    path. S matmuls/exps are column-restricted to the causal region;
    diagonal se tiles are dedicated per (pair, r) with their fully-masked
    left columns zeroed once at startup.
  - Head-pair S blocks land in one 2-bank psum tile => a single fused exp
    per (pair, ii) halves the ACT instruction count. ACT runs exp only;
    q/k/v psum copies are on DVE; softmax-sum broadcasts on gpsimd.
  - Emission is a deadline-driven schedule: every projection/out-projection
    matmul group becomes a filler with (deadline, ready) over a global unit
    order, flushed just-in-time or paced evenly, so the in-order PE queue
    never parks behind an exp-blocked score matmul. Out-projection drifts to
    the late (filler-starved) slices; the final slice normalizes in column
    chunks and alternates its psum copies between ACT and DVE to pipeline
    the tail stores.
  - Preload uses few large DMAs in consumption order (the DMA bus
    round-robins across queues, so small DMAs would let low-priority
    transfers steal bus turns from the critical first projection).
"""
import sys

import numpy as np

for _p in ("/opt/trn_rl_repo", "/root/.axon_site/_ro/trn_rl_repo"):
    if _p not in sys.path:
        try:
            import concourse  # noqa: F401
            break
        except ImportError:
            sys.path.append(_p)

import ml_dtypes  # noqa: E402
import concourse.bass as bass  # noqa: E402
import concourse.tile as tile  # noqa: E402
from concourse import bacc, mybir  # noqa: E402
from concourse.bass_utils import run_bass_kernel_spmd  # noqa: E402

P = 128
T = 2048
D = 1024
NH = 4          # heads per core
DH = 64
F = NH * DH     # per-core head features (256)
DC = D // P     # 8 contraction chunks
TJ = T // 512   # 4 tq slices
TC = T // P     # 16 tk chunks
N_CORES = 8
F32 = mybir.dt.float32
BF = mybir.dt.bfloat16
F8 = mybir.dt.float8e4
AF = mybir.ActivationFunctionType
DRM = mybir.MatmulPerfMode.DoubleRow
BFNP = ml_dtypes.bfloat16
F8NP = ml_dtypes.float8_e4m3

QK_FP8 = True  # q/k in fp8e4m3 + DoubleRow score matmul (2x PE on scores)

QK_DT = F8 if QK_FP8 else BF
WQ_SCALE = 32.0  # lifts Wq out of e4m3 subnormals; folded into exp scale
MASK_BIAS = -192.0 * WQ_SCALE  # exp(scale/WQ_SCALE * bias) = e^-24 ~ 4e-11


def build():
    nc = bacc.Bacc("TRN2", target_bir_lowering=False, debug=False, num_devices=N_CORES)
    xT = nc.dram_tensor("xT", [D, T], BF, kind="ExternalInput").ap()
    xT8 = nc.dram_tensor("xT8", [D, T], F8, kind="ExternalInput").ap()
    wqT = nc.dram_tensor("wqT", [D, F], F8, kind="ExternalInput").ap()
    wk8T = nc.dram_tensor("wk8T", [D // 2, F], F8, kind="ExternalInput").ap()
    wkT = nc.dram_tensor("wkT", [D // 2, F], BF, kind="ExternalInput").ap()
    wvT = nc.dram_tensor("wvT", [D, F], BF, kind="ExternalInput").ap()
    woT = nc.dram_tensor("woT", [F, D], BF, kind="ExternalInput").ap()
    ident = nc.dram_tensor("ident", [P, P], BF, kind="ExternalInput").ap()
    maskb = nc.dram_tensor("maskb", [P, P], BF, kind="ExternalInput").ap()
    out = nc.dram_tensor("out", [T, D], BF, kind="ExternalOutput").ap()

    scale = 1.0 / np.sqrt(DH) / (WQ_SCALE * WQ_SCALE)

    with tile.TileContext(nc) as tc:
        with (
            tc.tile_pool(name="weights", bufs=1) as wpool,
            tc.tile_pool(name="persist", bufs=1) as persist,
            tc.tile_pool(name="x", bufs=2) as xpool,
            tc.tile_pool(name="sexp", bufs=6) as sepool,
            tc.tile_pool(name="small", bufs=10) as small,
            tc.tile_pool(name="outsb", bufs=4) as opool,
            tc.tile_pool(name="ps_s", bufs=2, space="PSUM") as ps_s,
            tc.tile_pool(name="ps_y", bufs=2, space="PSUM") as ps_y,
            tc.tile_pool(name="ps_ao", bufs=2, space="PSUM") as ps_ao,
        ):
            wq_sb = wpool.tile([P, DC // 2, 2, F], F8)
            wk8_sb = wpool.tile([P, 2, 2, F], F8)
            wk_sb = wpool.tile([P, DC // 2, F], BF)
            wv_sb = wpool.tile([P, DC, F], BF)
            wo_sb = wpool.tile([P, 2, D], BF)
            mb_sb = wpool.tile([P, 1, P], BF)
            wq_r = wqT.rearrange("(o i p) f -> p o i f", p=P, i=2)
            wk8_r = wk8T.rearrange("(o i p) f -> p o i f", p=P, i=2)
            wk_r = wkT.rearrange("(o p) f -> p o f", p=P)
            wv_r = wvT.rearrange("(o p) f -> p o f", p=P)
            wo_r = woT.rearrange("(g p) e -> p g e", p=P)
            xT_r = xT.rearrange("(o p) t -> p o t", p=P)
            xT8_r = xT8.rearrange("(o i p) t -> p o i t", p=P, i=2)

            qT_sb = persist.tile([P, 2, T], QK_DT)
            kT_sb = persist.tile([P, 2, T], QK_DT)
            v_sb = persist.tile([P, NH, TC, DH + 1], BF)
            yT_sb = persist.tile([P, 2, T], BF)
            # dedicated diagonal se tiles per (pair c, r>=1): left cols hold
            # persistent zeros written once below.
            sediag = {
                (c, r): persist.tile([P, 2, 512], BF, name=f"sed_{c}_{r}")
                for c in range(2)
                for r in range(1, 4)
            }

            x_tiles = {}
            x8_tiles = {}

            def load_x(j):
                x_sb = xpool.tile([P, DC, 512], BF, tag="x", name=f"x_{j}")
                nc.sync.dma_start(x_sb[:], xT_r[:, :, 512 * j : 512 * (j + 1)])
                x_tiles[j] = x_sb
                x8_sb = xpool.tile([P, DC // 2, 2, 512], F8, tag="x8", name=f"x8_{j}")
                nc.sync.dma_start(x8_sb[:], xT8_r[:, :, :, 512 * j : 512 * (j + 1)])
                x8_tiles[j] = x8_sb

            # Preload in consumption order, few large DMAs: the DMA bus
            # round-robins across queues, so many small DMAs let low-priority
            # transfers steal bus turns from the critical first projection.
            x0_sb = xpool.tile([P, DC, 512], BF, tag="x", name="x_0")
            x_tiles[0] = x0_sb
            x80_sb = xpool.tile([P, DC // 2, 2, 512], F8, tag="x8", name="x8_0")
            x8_tiles[0] = x80_sb
            nc.sync.dma_start(wq_sb[:], wq_r[:])
            nc.sync.dma_start(x80_sb[:, 0:2], xT8_r[:, 0:2, :, 0:512])
            nc.sync.dma_start(x80_sb[:, 2:4], xT8_r[:, 2:4, :, 0:512])
            nc.sync.dma_start(wk8_sb[:], wk8_r[:])
            nc.sync.dma_start(wk_sb[:], wk_r[:])
            nc.sync.dma_start(x0_sb[:, 4:8], xT_r[:, 4:8, 0:512])
            nc.sync.dma_start(wv_sb[:], wv_r[:])
            nc.sync.dma_start(x0_sb[:, 0:4], xT_r[:, 0:4, 0:512])
            nc.sync.dma_start(mb_sb[:, 0], maskb)
            load_x(1)
            nc.sync.dma_start(wo_sb[:], wo_r[:])

            # V's softmax-sum ones column + persistent zeros in the
            # fully-masked left region of diagonal se tiles.
            nc.gpsimd.memset(v_sb[:, :, :, DH : DH + 1], 1.0)
            for (c, r), t_ in sediag.items():
                nc.gpsimd.memset(t_[:, :, 0 : 128 * r], 0.0)

            def proj_groups(j):
                jsl = slice(512 * j, 512 * (j + 1))
                x_sb = x_tiles[j]
                groups = []
                x8_sb = x8_tiles[j]
                for c in range(2):
                    def g(c=c):
                        pt = ps_ao.tile([P, 512], F32, tag="ao")
                        for o in range(DC // 2):
                            nc.tensor.matmul(
                                pt[:],
                                wq_sb[:, o, :, 128 * c : 128 * (c + 1)],
                                x8_sb[:, o, :, :],
                                start=(o == 0),
                                stop=(o == DC // 2 - 1),
                                perf_mode=DRM,
                            )
                        nc.vector.tensor_copy(qT_sb[:, c, jsl], pt[:])
                    groups.append(g)
                for c in range(2):
                    def g(c=c):
                        pt = ps_ao.tile([P, 512], F32, tag="ao")
                        for o in range(2):
                            nc.tensor.matmul(
                                pt[:],
                                wk8_sb[:, o, :, 128 * c : 128 * (c + 1)],
                                x8_sb[:, o, :, :],
                                start=(o == 0),
                                stop=False,
                                perf_mode=DRM,
                            )
                        for o in range(4, DC):
                            nc.tensor.matmul(
                                pt[:],
                                wk_sb[:, o - 4, 128 * c : 128 * (c + 1)],
                                x_sb[:, o, :],
                                start=False,
                                stop=(o == DC - 1),
                            )
                        nc.vector.tensor_copy(kT_sb[:, c, jsl], pt[:])
                    groups.append(g)
                for i in range(4):
                    def g(i=i):
                        pt = ps_ao.tile([P, 512], F32, tag="ao")
                        for o in range(DC):
                            nc.tensor.matmul(
                                pt[:, :F],
                                x_sb[:, o, 128 * i : 128 * (i + 1)],
                                wv_sb[:, o, :],
                                start=(o == 0),
                                stop=(o == DC - 1),
                            )
                        nc.vector.tensor_copy(
                            v_sb[:, :, 4 * j + i, 0:DH],
                            pt[:, :F].rearrange("p (h d) -> p h d", h=NH),
                        )
                    groups.append(g)
                return groups

            def outproj_groups(j, copy_on_act=False):
                groups = []
                for tb in range(4 * j, 4 * (j + 1)):
                    for eb in range(2):
                        def g(tb=tb, eb=eb):
                            pt = ps_ao.tile([P, 512], F32, tag="ao")
                            for g2 in range(2):
                                nc.tensor.matmul(
                                    pt[:],
                                    yT_sb[:, g2, 128 * tb : 128 * (tb + 1)],
                                    wo_sb[:, g2, 512 * eb : 512 * (eb + 1)],
                                    start=(g2 == 0),
                                    stop=(g2 == 1),
                                )
                            osb = opool.tile([P, 512], BF, tag="osb")
                            if copy_on_act and (tb + eb) % 2 == 0:
                                nc.scalar.copy(osb[:], pt[:])
                            else:
                                nc.vector.tensor_copy(osb[:], pt[:])
                            nc.sync.dma_start(
                                out[128 * tb : 128 * (tb + 1), 512 * eb : 512 * (eb + 1)],
                                osb[:],
                            )
                        groups.append(g)
                return groups

            def attn_unit(j, c, ii, nii, ypair, mid):
                r = ii - 4 * j
                col0 = 128 * r if r > 0 else 0
                qsl = slice(512 * j + col0, 512 * (j + 1))
                diag = r >= 0
                sps = ps_s.tile([P, 2, 512], F32, tag="s")
                for t in range(2):
                    if QK_FP8:
                        h = 2 * c + t
                        nc.tensor.matmul(
                            sps[:, t, col0:],
                            kT_sb[32 * h : 32 * h + 32, :, 128 * ii : 128 * (ii + 1)],
                            qT_sb[32 * h : 32 * h + 32, :, qsl],
                            start=True,
                            stop=True,
                            perf_mode=DRM,
                            tile_position=(32 * h, 0),
                        )
                    else:
                        hp = 64 * t
                        nc.tensor.matmul(
                            sps[:, t, col0:],
                            kT_sb[hp : hp + DH, c, 128 * ii : 128 * (ii + 1)],
                            qT_sb[hp : hp + DH, c, qsl],
                            start=True,
                            stop=True,
                        )

                se = (
                    sediag[(c, r)]
                    if r >= 1
                    else sepool.tile([P, 2, 512], BF, tag="se")
                )
                nc.scalar.activation(
                    se[:, :, col0:], sps[:, :, col0:], AF.Exp, scale=scale
                )
                if diag:
                    # zero the strictly-upper part of the mixed 128-block
                    nc.vector.tensor_mul(
                        se[:, :, col0 : col0 + 128],
                        se[:, :, col0 : col0 + 128],
                        mb_sb[:, 0:1, :].to_broadcast([P, 2, P]),
                    )
                for f in mid:
                    f()
                for t in range(2):
                    nc.tensor.matmul(
                        ypair[t][:],
                        v_sb[:, 2 * c + t, ii, :],
                        se[:, t, :],
                        start=(ii == 0),
                        stop=(ii == nii - 1),
                    )

            def normalize(j, c, ypair):
                # All recips first (DVE), broadcasts next (Pool, overlaps the
                # recips), muls last — so the in-order DVE queue never parks
                # behind a Pool broadcast. Chunked on the last slice so
                # outproj(3) tb-group tb starts as soon as its yT chunk lands.
                nchunk = 2 if j == TJ - 1 else 1
                w = 512 // nchunk
                bsbs = {}
                for q4 in range(nchunk):
                    for t in range(2):
                        qs = slice(w * q4, w * (q4 + 1))
                        rec = small.tile([1, w], F32, tag=f"rec{nchunk}")
                        nc.vector.reciprocal(rec[:], ypair[t][DH : DH + 1, qs])
                        bsb = small.tile([DH, w], F32, tag=f"bsb{nchunk}")
                        nc.gpsimd.partition_broadcast(bsb[:], rec[:])
                        bsbs[(t, q4)] = bsb
                for q4 in range(nchunk):
                    for t in range(2):
                        qs = slice(w * q4, w * (q4 + 1))
                        ys = slice(512 * j + w * q4, 512 * j + w * (q4 + 1))
                        nc.vector.tensor_mul(
                            yT_sb[64 * t : 64 * t + DH, c, ys],
                            ypair[t][0:DH, qs],
                            bsbs[(t, q4)],
                        )

            # Remaining x slices: queued behind the preload on the same DMA
            # ring, arriving well before their projections need them.
            load_x(2)
            load_x(3)

            # ---- deadline-driven emission schedule ----
            # One global unit order; every proj/outproj matmul group becomes
            # a filler with (deadline unit, ready unit). Deadline fillers are
            # flushed right before their unit; the rest pace out evenly so
            # the in-order PE queue always has independent work between
            # exp-blocked attention steps. proj(0) interleaves into the j=0
            # units, so the ACT exp pipeline starts ~14us earlier.
            sched = []
            for j in range(TJ):
                for c in range(2):
                    for ii in range(4 * j + 4):
                        sched.append((j, c, ii))
            idx = {u: s for s, u in enumerate(sched)}
            NU = len(sched)

            fillers = []  # (deadline, ready, fn, pe_cost)
            QC, KC, VC, OC = 1024, 2560, 4096, 1024  # PE cycles per group
            for j in range(TJ):
                pg = proj_groups(j)  # [q-c0, q-c1, k-c0, k-c1, v0..v3]
                for c in range(2):
                    # fp8 layout: dim1 of qT/kT is the dh-tile, so every S
                    # matmul reads BOTH projection chunks => both must land
                    # before the slice's first unit.
                    d = idx[(j, 0, 0)] if QK_FP8 else idx[(j, c, 0)]
                    fillers.append((d, 0, pg[c], QC))
                    fillers.append((d, 0, pg[2 + c], KC))
                for i4 in range(4):
                    fillers.append((idx[(j, 0, 4 * j + i4)], 0, pg[4 + i4], VC))
            for j in range(TJ - 1):
                ready = idx[(j, 1, 4 * j + 3)] + 1
                for g in outproj_groups(j):
                    fillers.append((NU, ready, g, OC))
            fillers.sort(key=lambda f: (f[0], f[1]))
            NF = len(fillers)
            total_cost = sum(f[3] for f in fillers)

            fi = 0
            done_cost = 0
            ypairs = {}
            for s, (j, c, ii) in enumerate(sched):
                nii = 4 * j + 4
                while fi < NF and fillers[fi][0] <= s:
                    done_cost += fillers[fi][3]
                    fillers[fi][2]()
                    fi += 1
                if ii == 0:
                    ypairs[c] = [
                        ps_y.tile([DH + 1, 512], F32, tag="y", name=f"y_{j}_{c}_{t}")
                        for t in range(2)
                    ]
                attn_unit(j, c, ii, nii, ypairs[c], [])
                if ii == nii - 1:
                    normalize(j, c, ypairs[c])
                # pace by PE-time, not group count: a v-group is 4x a q-group
                target = (s + 1) * total_cost // NU
                while fi < NF and done_cost < target and fillers[fi][1] <= s:
                    done_cost += fillers[fi][3]
                    fillers[fi][2]()
                    fi += 1
            while fi < NF:
                fillers[fi][2]()
                fi += 1
            for g in outproj_groups(TJ - 1, copy_on_act=True):
                g()
    nc.compile()
    return nc


def make_ident() -> np.ndarray:
    return np.eye(P, dtype=np.float32)


def make_maskb() -> np.ndarray:
    q = np.arange(P)[None, :]
    p = np.arange(P)[:, None]
    return np.ascontiguousarray((q >= p).astype(np.float32))


def qk_col_perm() -> np.ndarray:
    """Column permutation of wq/wk so the projection psum partitions land in
    the fp8 DoubleRow [32, 2] packing: matmul chunk c, out partition
    p = 32*h + p'  <->  feature f = 64*h + 32*c + p'."""
    perm = []
    for c in range(2):
        for jj in range(128):
            perm.append(64 * (jj // 32) + 32 * c + (jj % 32))
    return np.asarray(perm)


def shard_inputs(x, Wqkv, Wout):
    ident = make_ident().astype(BFNP)
    maskb = make_maskb().astype(BFNP)
    perm = qk_col_perm() if QK_FP8 else np.arange(F)
    in_maps = []
    for core in range(N_CORES):
        b, g = core // 4, core % 4
        sl = slice(F * g, F * (g + 1))
        wq = np.ascontiguousarray(Wqkv[sl, :].T[:, perm])
        wk = np.ascontiguousarray(Wqkv[D:][sl, :].T[:, perm])
        in_maps.append(
            {
                "xT": np.ascontiguousarray(x[b].T).astype(BFNP),
                "xT8": np.ascontiguousarray(x[b].T).astype(F8NP),
                "wqT": (wq * WQ_SCALE).astype(F8NP),
                "wk8T": (wk[:D // 2] * WQ_SCALE).astype(F8NP),
                "wkT": (wk[D // 2:] * WQ_SCALE).astype(BFNP),
                "wvT": np.ascontiguousarray(Wqkv[2 * D:][sl, :].T).astype(BFNP),
                "woT": np.ascontiguousarray(Wout[:, sl].T).astype(BFNP),
                "ident": ident,
                "maskb": maskb,
            }
        )
    return in_maps


_NC_CACHE = None


def kernel(x, Wqkv, Wout):
    global _NC_CACHE
    x = np.asarray(x, dtype=np.float32)
    Wqkv = np.asarray(Wqkv, dtype=np.float32)
    Wout = np.asarray(Wout, dtype=np.float32)
    if _NC_CACHE is None:
        _NC_CACHE = build()
    nc = _NC_CACHE
    in_maps = shard_inputs(x, Wqkv, Wout)
    res = run_bass_kernel_spmd(nc, in_maps, core_ids=list(range(N_CORES)))
    outs = [res.results[c]["out"].astype(np.float32) for c in range(N_CORES)]
    return np.stack(
        [outs[0] + outs[1] + outs[2] + outs[3], outs[4] + outs[5] + outs[6] + outs[7]]
    )


# revision 44
# speedup vs baseline: 1.0606x; 1.0035x over previous
"""Multi-head causal self-attention (B=2, T=2048, D=1024, H=16, Dh=64) on 8 TRN2 cores.

Sharding: data-parallel over batch (2 groups of 4 cores), tensor-parallel over
heads within a group (4 heads/core). Each core computes its 4 heads'
QKV projection + causal flash attention + its slice of the output projection;
the host sums the 4 partial outputs per batch (bf16 partials, f32 sum).

Design (per core):
  - All matmuls bf16 (1 cyc/row at every N; fp32r pays 4x below N=256),
    except: (a) the score matmul - q/k are stored fp8e4m3 in a [32,2]
    DoubleRow packing, so S^T runs at 0.5 cyc/row (2x PE); (b) the
    q-projection itself - x and Wq ship as fp8 and contract DoubleRow
    (4 chunks of K=256), another 2x. Wq is pre-scaled by 32 on the host to
    lift it out of e4m3's subnormal range (compensated in the exp scale;
    the mask bias scales to -6144). The wq/wk columns are pre-permuted on
    the host so the projection psum lands directly in the packed S layout
    (the psum->sbuf copy is partition-preserving). k/v/out projections and
    PV stay bf16 - fp8 there fails the 2e-2 gate. Measured rel-err ~1.4e-2.
  - Attention runs transposed (S^T[tk, tq] blocks, 512-wide tq slices);
    softmax sums come free via a ones-column appended to V, so PV emits
    [y^T; sums] per head with no extra matmul cost (cost = moving dim only).
  - Causal mask is an additive bias matmul (identity lhsT x [-192 staircase]
    bf16) into the score psum on the mixed diagonal 128-block only; exp then
    yields e^-24 ~ 0 there. No vector-engine work on the exp->PV critical
    path. S matmuls/exps are column-restricted to the causal region;
    diagonal se tiles are dedicated per (pair, r) with their fully-masked
    left columns zeroed once at startup.
  - Head-pair S blocks land in one 2-bank psum tile => a single fused exp
    per (pair, ii) halves the ACT instruction count. ACT runs exp only;
    q/k/v psum copies are on DVE; softmax-sum broadcasts on gpsimd.
  - Emission is a deadline-driven schedule: every projection/out-projection
    matmul group becomes a filler with (deadline, ready) over a global unit
    order, flushed just-in-time or paced evenly, so the in-order PE queue
    never parks behind an exp-blocked score matmul. Out-projection drifts to
    the late (filler-starved) slices; the final slice normalizes in column
    chunks and alternates its psum copies between ACT and DVE to pipeline
    the tail stores.
  - Preload uses few large DMAs in consumption order (the DMA bus
    round-robins across queues, so small DMAs would let low-priority
    transfers steal bus turns from the critical first projection).
"""
import sys

import numpy as np

for _p in ("/opt/trn_rl_repo", "/root/.axon_site/_ro/trn_rl_repo"):
    if _p not in sys.path:
        try:
            import concourse  # noqa: F401
            break
        except ImportError:
            sys.path.append(_p)

import ml_dtypes  # noqa: E402
import concourse.bass as bass  # noqa: E402
import concourse.tile as tile  # noqa: E402
from concourse import bacc, mybir  # noqa: E402
from concourse.bass_utils import run_bass_kernel_spmd  # noqa: E402

P = 128
T = 2048
D = 1024
NH = 4          # heads per core
DH = 64
F = NH * DH     # per-core head features (256)
DC = D // P     # 8 contraction chunks
TJ = T // 512   # 4 tq slices
TC = T // P     # 16 tk chunks
N_CORES = 8
F32 = mybir.dt.float32
BF = mybir.dt.bfloat16
F8 = mybir.dt.float8e4
AF = mybir.ActivationFunctionType
DRM = mybir.MatmulPerfMode.DoubleRow
BFNP = ml_dtypes.bfloat16
F8NP = ml_dtypes.float8_e4m3

QK_FP8 = True  # q/k in fp8e4m3 + DoubleRow score matmul (2x PE on scores)

QK_DT = F8 if QK_FP8 else BF
WQ_SCALE = 32.0  # lifts Wq out of e4m3 subnormals; folded into exp scale
MASK_BIAS = -192.0 * WQ_SCALE  # exp(scale/WQ_SCALE * bias) = e^-24 ~ 4e-11


def build():
    nc = bacc.Bacc("TRN2", target_bir_lowering=False, debug=False, num_devices=N_CORES)
    xT = nc.dram_tensor("xT", [D, T], BF, kind="ExternalInput").ap()
    xT8 = nc.dram_tensor("xT8", [D, T], F8, kind="ExternalInput").ap()
    wqT = nc.dram_tensor("wqT", [D, F], F8, kind="ExternalInput").ap()
    wk8T = nc.dram_tensor("wk8T", [D // 2, F], F8, kind="ExternalInput").ap()
    wkT = nc.dram_tensor("wkT", [D // 2, F], BF, kind="ExternalInput").ap()
    wvT = nc.dram_tensor("wvT", [D, F], BF, kind="ExternalInput").ap()
    woT = nc.dram_tensor("woT", [F, D], BF, kind="ExternalInput").ap()
    ident = nc.dram_tensor("ident", [P, P], BF, kind="ExternalInput").ap()
    maskb = nc.dram_tensor("maskb", [P, P], BF, kind="ExternalInput").ap()
    out = nc.dram_tensor("out", [T, D], BF, kind="ExternalOutput").ap()

    scale = 1.0 / np.sqrt(DH) / (WQ_SCALE * WQ_SCALE)

    with tile.TileContext(nc) as tc:
        with (
            tc.tile_pool(name="weights", bufs=1) as wpool,
            tc.tile_pool(name="persist", bufs=1) as persist,
            tc.tile_pool(name="x", bufs=2) as xpool,
            tc.tile_pool(name="sexp", bufs=8) as sepool,
            tc.tile_pool(name="small", bufs=10) as small,
            tc.tile_pool(name="outsb", bufs=8) as opool,
            tc.tile_pool(name="ps_s", bufs=2, space="PSUM") as ps_s,
            tc.tile_pool(name="ps_y", bufs=2, space="PSUM") as ps_y,
            tc.tile_pool(name="ps_ao", bufs=2, space="PSUM") as ps_ao,
        ):
            wq_sb = wpool.tile([P, DC // 2, 2, F], F8)
            wk8_sb = wpool.tile([P, 2, 2, F], F8)
            wk_sb = wpool.tile([P, DC // 2, F], BF)
            wv_sb = wpool.tile([P, DC, F], BF)
            wo_sb = wpool.tile([P, 2, D], BF)
            mb_sb = wpool.tile([P, 1, P], BF)
            wq_r = wqT.rearrange("(o i p) f -> p o i f", p=P, i=2)
            wk8_r = wk8T.rearrange("(o i p) f -> p o i f", p=P, i=2)
            wk_r = wkT.rearrange("(o p) f -> p o f", p=P)
            wv_r = wvT.rearrange("(o p) f -> p o f", p=P)
            wo_r = woT.rearrange("(g p) e -> p g e", p=P)
            xT_r = xT.rearrange("(o p) t -> p o t", p=P)
            xT8_r = xT8.rearrange("(o i p) t -> p o i t", p=P, i=2)

            qT_sb = persist.tile([P, 2, T], QK_DT)
            kT_sb = persist.tile([P, 2, T], QK_DT)
            v_sb = persist.tile([P, NH, TC, DH + 1], BF)
            yT_sb = persist.tile([P, 2, T], BF)
            # dedicated diagonal se tiles per (pair c, r>=1): left cols hold
            # persistent zeros written once below.
            sediag = {
                (c, r): persist.tile([P, 2, 512], BF, name=f"sed_{c}_{r}")
                for c in range(2)
                for r in range(1, 4)
            }

            x_tiles = {}
            x8_tiles = {}

            def load_x(j):
                # k-proj's bf16 half reads only chunks 4-7; ship those first
                x_sb = xpool.tile([P, DC, 512], BF, tag="x", name=f"x_{j}")
                nc.sync.dma_start(x_sb[:, 4:8], xT_r[:, 4:8, 512 * j : 512 * (j + 1)])
                nc.sync.dma_start(x_sb[:, 0:4], xT_r[:, 0:4, 512 * j : 512 * (j + 1)])
                x_tiles[j] = x_sb
                x8_sb = xpool.tile([P, DC // 2, 2, 512], F8, tag="x8", name=f"x8_{j}")
                nc.sync.dma_start(x8_sb[:], xT8_r[:, :, :, 512 * j : 512 * (j + 1)])
                x8_tiles[j] = x8_sb

            # Preload in consumption order, few large DMAs: the DMA bus
            # round-robins across queues, so many small DMAs let low-priority
            # transfers steal bus turns from the critical first projection.
            x0_sb = xpool.tile([P, DC, 512], BF, tag="x", name="x_0")
            x_tiles[0] = x0_sb
            x80_sb = xpool.tile([P, DC // 2, 2, 512], F8, tag="x8", name="x8_0")
            x8_tiles[0] = x80_sb
            nc.sync.dma_start(wq_sb[:], wq_r[:])
            nc.sync.dma_start(x80_sb[:, 0:2], xT8_r[:, 0:2, :, 0:512])
            nc.sync.dma_start(x80_sb[:, 2:4], xT8_r[:, 2:4, :, 0:512])
            nc.sync.dma_start(wk8_sb[:], wk8_r[:])
            nc.sync.dma_start(wk_sb[:], wk_r[:])
            nc.sync.dma_start(x0_sb[:, 4:8], xT_r[:, 4:8, 0:512])
            nc.sync.dma_start(wv_sb[:], wv_r[:])
            nc.sync.dma_start(x0_sb[:, 0:4], xT_r[:, 0:4, 0:512])
            nc.sync.dma_start(mb_sb[:, 0], maskb)
            load_x(1)
            nc.sync.dma_start(wo_sb[:], wo_r[:])

            # V's softmax-sum ones column + persistent zeros in the
            # fully-masked left region of diagonal se tiles.
            nc.gpsimd.memset(v_sb[:, :, :, DH : DH + 1], 1.0)
            for (c, r), t_ in sediag.items():
                nc.gpsimd.memset(t_[:, :, 0 : 128 * r], 0.0)

            def proj_groups(j):
                jsl = slice(512 * j, 512 * (j + 1))
                x_sb = x_tiles[j]
                groups = []
                x8_sb = x8_tiles[j]
                for c in range(2):
                    def g(c=c):
                        pt = ps_ao.tile([P, 512], F32, tag="ao")
                        for o in range(DC // 2):
                            nc.tensor.matmul(
                                pt[:],
                                wq_sb[:, o, :, 128 * c : 128 * (c + 1)],
                                x8_sb[:, o, :, :],
                                start=(o == 0),
                                stop=(o == DC // 2 - 1),
                                perf_mode=DRM,
                            )
                        nc.vector.tensor_copy(qT_sb[:, c, jsl], pt[:])
                    groups.append(g)
                for c in range(2):
                    def g(c=c):
                        pt = ps_ao.tile([P, 512], F32, tag="ao")
                        for o in range(2):
                            nc.tensor.matmul(
                                pt[:],
                                wk8_sb[:, o, :, 128 * c : 128 * (c + 1)],
                                x8_sb[:, o, :, :],
                                start=(o == 0),
                                stop=False,
                                perf_mode=DRM,
                            )
                        for o in range(4, DC):
                            nc.tensor.matmul(
                                pt[:],
                                wk_sb[:, o - 4, 128 * c : 128 * (c + 1)],
                                x_sb[:, o, :],
                                start=False,
                                stop=(o == DC - 1),
                            )
                        nc.vector.tensor_copy(kT_sb[:, c, jsl], pt[:])
                    groups.append(g)
                for i in range(4):
                    def g(i=i):
                        pt = ps_ao.tile([P, 512], F32, tag="ao")
                        for o in range(DC):
                            nc.tensor.matmul(
                                pt[:, :F],
                                x_sb[:, o, 128 * i : 128 * (i + 1)],
                                wv_sb[:, o, :],
                                start=(o == 0),
                                stop=(o == DC - 1),
                            )
                        nc.vector.tensor_copy(
                            v_sb[:, :, 4 * j + i, 0:DH],
                            pt[:, :F].rearrange("p (h d) -> p h d", h=NH),
                        )
                    groups.append(g)
                return groups

            def outproj_groups(j, copy_on_act=False):
                groups = []
                for tb in range(4 * j, 4 * (j + 1)):
                    for eb in range(2):
                        def g(tb=tb, eb=eb):
                            pt = ps_ao.tile([P, 512], F32, tag="ao")
                            for g2 in range(2):
                                nc.tensor.matmul(
                                    pt[:],
                                    yT_sb[:, g2, 128 * tb : 128 * (tb + 1)],
                                    wo_sb[:, g2, 512 * eb : 512 * (eb + 1)],
                                    start=(g2 == 0),
                                    stop=(g2 == 1),
                                )
                            osb = opool.tile([P, 512], BF, tag="osb")
                            if copy_on_act and (tb + eb) % 2 == 0:
                                nc.scalar.copy(osb[:], pt[:])
                            else:
                                nc.vector.tensor_copy(osb[:], pt[:])
                            nc.sync.dma_start(
                                out[128 * tb : 128 * (tb + 1), 512 * eb : 512 * (eb + 1)],
                                osb[:],
                            )
                        groups.append(g)
                return groups

            def attn_unit(j, c, ii, nii, ypair, mid):
                r = ii - 4 * j
                col0 = 128 * r if r > 0 else 0
                qsl = slice(512 * j + col0, 512 * (j + 1))
                diag = r >= 0
                sps = ps_s.tile([P, 2, 512], F32, tag="s")
                for t in range(2):
                    if QK_FP8:
                        h = 2 * c + t
                        nc.tensor.matmul(
                            sps[:, t, col0:],
                            kT_sb[32 * h : 32 * h + 32, :, 128 * ii : 128 * (ii + 1)],
                            qT_sb[32 * h : 32 * h + 32, :, qsl],
                            start=True,
                            stop=True,
                            perf_mode=DRM,
                            tile_position=(32 * h, 0),
                        )
                    else:
                        hp = 64 * t
                        nc.tensor.matmul(
                            sps[:, t, col0:],
                            kT_sb[hp : hp + DH, c, 128 * ii : 128 * (ii + 1)],
                            qT_sb[hp : hp + DH, c, qsl],
                            start=True,
                            stop=True,
                        )

                se = (
                    sediag[(c, r)]
                    if r >= 1
                    else sepool.tile([P, 2, 512], BF, tag="se")
                )
                nc.scalar.activation(
                    se[:, :, col0:], sps[:, :, col0:], AF.Exp, scale=scale
                )
                if diag:
                    # zero the strictly-upper part of the mixed 128-block
                    nc.vector.tensor_mul(
                        se[:, :, col0 : col0 + 128],
                        se[:, :, col0 : col0 + 128],
                        mb_sb[:, 0:1, :].to_broadcast([P, 2, P]),
                    )
                for f in mid:
                    f()
                for t in range(2):
                    nc.tensor.matmul(
                        ypair[t][:],
                        v_sb[:, 2 * c + t, ii, :],
                        se[:, t, :],
                        start=(ii == 0),
                        stop=(ii == nii - 1),
                    )

            def normalize(j, c, ypair):
                # All recips first (DVE), broadcasts next (Pool, overlaps the
                # recips), muls last — so the in-order DVE queue never parks
                # behind a Pool broadcast. Chunked on the last slice so
                # outproj(3) tb-group tb starts as soon as its yT chunk lands.
                nchunk = 2 if j == TJ - 1 else 1
                w = 512 // nchunk
                bsbs = {}
                for q4 in range(nchunk):
                    for t in range(2):
                        qs = slice(w * q4, w * (q4 + 1))
                        rec = small.tile([1, w], F32, tag=f"rec{nchunk}")
                        nc.vector.reciprocal(rec[:], ypair[t][DH : DH + 1, qs])
                        bsb = small.tile([DH, w], F32, tag=f"bsb{nchunk}")
                        nc.gpsimd.partition_broadcast(bsb[:], rec[:])
                        bsbs[(t, q4)] = bsb
                for q4 in range(nchunk):
                    for t in range(2):
                        qs = slice(w * q4, w * (q4 + 1))
                        ys = slice(512 * j + w * q4, 512 * j + w * (q4 + 1))
                        nc.vector.tensor_mul(
                            yT_sb[64 * t : 64 * t + DH, c, ys],
                            ypair[t][0:DH, qs],
                            bsbs[(t, q4)],
                        )

            # Remaining x slices: queued behind the preload on the same DMA
            # ring, arriving well before their projections need them.
            load_x(2)
            load_x(3)

            # ---- deadline-driven emission schedule ----
            # One global unit order; every proj/outproj matmul group becomes
            # a filler with (deadline unit, ready unit). Deadline fillers are
            # flushed right before their unit; the rest pace out evenly so
            # the in-order PE queue always has independent work between
            # exp-blocked attention steps. proj(0) interleaves into the j=0
            # units, so the ACT exp pipeline starts ~14us earlier.
            sched = []
            for j in range(TJ):
                for c in range(2):
                    for ii in range(4 * j + 4):
                        sched.append((j, c, ii))
            idx = {u: s for s, u in enumerate(sched)}
            NU = len(sched)

            fillers = []  # (deadline, ready, fn, pe_cost)
            QC, KC, VC, OC = 1024, 2560, 4096, 1024  # PE cycles per group
            for j in range(TJ):
                pg = proj_groups(j)  # [q-c0, q-c1, k-c0, k-c1, v0..v3]
                for c in range(2):
                    # fp8 layout: dim1 of qT/kT is the dh-tile, so every S
                    # matmul reads BOTH projection chunks => both must land
                    # before the slice's first unit.
                    d = idx[(j, 0, 0)] if QK_FP8 else idx[(j, c, 0)]
                    fillers.append((d, 0, pg[c], QC))
                    fillers.append((d, 0, pg[2 + c], KC))
                for i4 in range(4):
                    fillers.append((idx[(j, 0, 4 * j + i4)], 0, pg[4 + i4], VC))
            for j in range(TJ - 1):
                ready = idx[(j, 1, 4 * j + 3)] + 1
                for g in outproj_groups(j):
                    fillers.append((NU, ready, g, OC))
            fillers.sort(key=lambda f: (f[0], f[1]))
            NF = len(fillers)
            total_cost = sum(f[3] for f in fillers)

            fi = 0
            done_cost = 0
            ypairs = {}
            for s, (j, c, ii) in enumerate(sched):
                nii = 4 * j + 4
                while fi < NF and fillers[fi][0] <= s:
                    done_cost += fillers[fi][3]
                    fillers[fi][2]()
                    fi += 1
                if ii == 0:
                    ypairs[c] = [
                        ps_y.tile([DH + 1, 512], F32, tag="y", name=f"y_{j}_{c}_{t}")
                        for t in range(2)
                    ]
                attn_unit(j, c, ii, nii, ypairs[c], [])
                if ii == nii - 1:
                    normalize(j, c, ypairs[c])
                # pace by PE-time, not group count: a v-group is 4x a q-group
                target = (s + 1) * total_cost // NU
                while fi < NF and done_cost < target and fillers[fi][1] <= s:
                    done_cost += fillers[fi][3]
                    fillers[fi][2]()
                    fi += 1
            while fi < NF:
                fillers[fi][2]()
                fi += 1
            for g in outproj_groups(TJ - 1, copy_on_act=True):
                g()
    nc.compile()
    return nc


def make_ident() -> np.ndarray:
    return np.eye(P, dtype=np.float32)


def make_maskb() -> np.ndarray:
    q = np.arange(P)[None, :]
    p = np.arange(P)[:, None]
    return np.ascontiguousarray((q >= p).astype(np.float32))


def qk_col_perm() -> np.ndarray:
    """Column permutation of wq/wk so the projection psum partitions land in
    the fp8 DoubleRow [32, 2] packing: matmul chunk c, out partition
    p = 32*h + p'  <->  feature f = 64*h + 32*c + p'."""
    perm = []
    for c in range(2):
        for jj in range(128):
            perm.append(64 * (jj // 32) + 32 * c + (jj % 32))
    return np.asarray(perm)


def shard_inputs(x, Wqkv, Wout):
    ident = make_ident().astype(BFNP)
    maskb = make_maskb().astype(BFNP)
    perm = qk_col_perm() if QK_FP8 else np.arange(F)
    in_maps = []
    for core in range(N_CORES):
        b, g = core // 4, core % 4
        sl = slice(F * g, F * (g + 1))
        wq = np.ascontiguousarray(Wqkv[sl, :].T[:, perm])
        wk = np.ascontiguousarray(Wqkv[D:][sl, :].T[:, perm])
        in_maps.append(
            {
                "xT": np.ascontiguousarray(x[b].T).astype(BFNP),
                "xT8": np.ascontiguousarray(x[b].T).astype(F8NP),
                "wqT": (wq * WQ_SCALE).astype(F8NP),
                "wk8T": (wk[:D // 2] * WQ_SCALE).astype(F8NP),
                "wkT": (wk[D // 2:] * WQ_SCALE).astype(BFNP),
                "wvT": np.ascontiguousarray(Wqkv[2 * D:][sl, :].T).astype(BFNP),
                "woT": np.ascontiguousarray(Wout[:, sl].T).astype(BFNP),
                "ident": ident,
                "maskb": maskb,
            }
        )
    return in_maps


_NC_CACHE = None


def kernel(x, Wqkv, Wout):
    global _NC_CACHE
    x = np.asarray(x, dtype=np.float32)
    Wqkv = np.asarray(Wqkv, dtype=np.float32)
    Wout = np.asarray(Wout, dtype=np.float32)
    if _NC_CACHE is None:
        _NC_CACHE = build()
    nc = _NC_CACHE
    in_maps = shard_inputs(x, Wqkv, Wout)
    res = run_bass_kernel_spmd(nc, in_maps, core_ids=list(range(N_CORES)))
    outs = [res.results[c]["out"].astype(np.float32) for c in range(N_CORES)]
    return np.stack(
        [outs[0] + outs[1] + outs[2] + outs[3], outs[4] + outs[5] + outs[6] + outs[7]]
    )
